# revision 1
# baseline (speedup 1.0000x reference)
"""Trainium2 Bass kernel for nn_Caption (bidirectional-LSTM image captioner).

Distribution over 8 NeuronCores (zero per-step collectives):
  - Recurrent computation (both LSTM layers, lin, context attention) is
    REPLICATED on all cores with the full batch of 64: per-step gate matmuls
    are PE-streaming-bound (cost independent of batch <= 128), so replication
    is free and avoids per-step collectives (AllGather floor ~5us x 24 steps).
  - Vocab projection (12000) is sharded 8-way (1500 cols/core).
  - The 1x1 conv ("mapped") is sharded by batch (8 rows/core) and AllGathered
    once (fp16) at the start; every core holds the full mapped for the
    per-step context matvecs.
  - log_softmax: logits are tiny so no max-subtraction is needed; each core
    accumulates per-(t,n) sum of exp over its vocab slice; ONE AllReduce of
    (64,24) sums at the end; final pass writes x - ln(s_global).

Layout: all matmuls are activation-stationary (lhsT = activations^T), so
activations are transposed each step via PE transposes.  Biases ride as
extra contraction rows against constant-1 rows in the transposed
activations.  sigma(x)=0.5*tanh(x/2)+0.5 with the 0.5 pre-scaled into the
i/f/o weight columns so one plain tanh covers all gates.  Cell state is kept
scaled (Ct=2c, h~=2h) with 0.5 folded into downstream weights; the
l2-normalized ctx is invariant to the h~ scaling.
"""

import sys
import numpy as np

for _p in ("/opt/trn_rl_repo",):
    if _p not in sys.path:
        sys.path.insert(0, _p)

import concourse.bass as bass
import concourse.tile as tile
from concourse import bacc
from concourse import mybir
from concourse.masks import make_identity
from concourse.bass_utils import run_bass_kernel_spmd

F16 = mybir.dt.float16
F8 = mybir.dt.float8e4
F32 = mybir.dt.float32
I32 = mybir.dt.int32
AF = mybir.ActivationFunctionType
OP = mybir.AluOpType

N = 64          # batch
T = 24          # steps
E = 196         # embedding/hidden size
M = 512         # context dim
C = 2048        # image channels
V = 12000       # vocab
NCORES = 8
VS = V // NCORES          # vocab slice per core
NL = N // NCORES          # batch rows per core (conv shard)
NS = NL * E               # conv rows per core (1568)
G2 = 2 * 4 * E            # gate cols, both dirs (1568)
RG = [list(range(NCORES))]
GNT = 392                 # gates N-tile
VOC_NT = [(0, 512), (512, 512), (1024, 476)]
LRAW_W = 1536             # padded row width of raw-logit staging

# h^T tiles are blocked {128, 68, 128, 68(+ones)} so fwd/bwd chunks align.
HBLK = [(0, 128), (128, 68), (196, 128), (324, 68)]


def _f16(x):
    return np.ascontiguousarray(x, dtype=np.float16)


def _f32(x):
    return np.ascontiguousarray(x, dtype=np.float32)


def prepare_inputs(inputs):
    img = _f32(np.asarray(inputs["input_image_feat"])).reshape(N, E, C)
    seq = np.ascontiguousarray(np.asarray(inputs["sequences"]).astype(np.int32))
    conv_w = _f32(inputs["conv_w"]); conv_b = _f32(inputs["conv_b"])
    fcg_w = _f32(inputs["fcg_w"]); fcg_b = _f32(inputs["fcg_b"])
    emb = _f32(inputs["emb"])
    w_ih0 = _f32(inputs["w_ih0"]); w_hh0 = _f32(inputs["w_hh0"]); b0 = _f32(inputs["b0"])
    w_ih1 = _f32(inputs["w_ih1"]); w_hh1 = _f32(inputs["w_hh1"]); b1 = _f32(inputs["b1"])
    lin_w = _f32(inputs["lin_w"]); lin_b = _f32(inputs["lin_b"])
    wp_w = _f32(inputs["wp_w"]); wp_b = _f32(inputs["wp_b"])

    # gate reorder [i f g o] -> [i f o g]; pre-scale i/f/o columns by 0.5
    perm = np.r_[0:E, E:2 * E, 3 * E:4 * E, 2 * E:3 * E]
    gsc = np.ones(4 * E, np.float32)
    gsc[: 3 * E] = 0.5

    def gmat(w):            # (784, in) -> (in, 784) permuted + scaled
        return w.T[:, perm] * gsc

    def gvec(b):
        return b[perm] * gsc

    W0 = np.concatenate([gmat(w_ih0[0]), gmat(w_ih0[1])], axis=1)        # (708,1568)
    b0r = np.concatenate([gvec(b0[0]), gvec(b0[1])])
    W0e = _f16(np.concatenate([W0[:E], b0r[None]], axis=0))              # (197,1568)
    W0c = _f16(W0[E:E + M])                                              # (512,1568)
    W0h = _f16(0.5 * np.concatenate([gmat(w_hh0[0]), gmat(w_hh0[1])], 1))  # (196,1568)
    W1 = 0.5 * np.concatenate([gmat(w_ih1[0]), gmat(w_ih1[1])], axis=1)  # (392,1568)
    b1r = np.concatenate([gvec(b1[0]), gvec(b1[1])])
    W1x = _f16(np.concatenate([W1, b1r[None]], axis=0))                  # (393,1568)
    W1h = _f16(0.5 * np.concatenate([gmat(w_hh1[0]), gmat(w_hh1[1])], 1))  # (196,1568)
    lin_aug = _f16(np.concatenate(                                       # (905,512)
        [0.5 * lin_w.T[:2 * E], lin_b[None], lin_w.T[2 * E:]], axis=0))
    conv_wT_aug = _f16(np.concatenate([conv_w.T, conv_b[None]], axis=0))  # (2049,512)

    base = dict(
        W0e=W0e, W0c=W0c, W0h=W0h, W1x=W1x, W1h=W1h, lin_aug=lin_aug,
        conv_wT_aug=conv_wT_aug, fcg_wT=_f16(fcg_w.T),
        fcg_b=_f32(fcg_b.reshape(E, 1)), emb=emb,
        seq_idx=np.ascontiguousarray(seq.reshape(T * N, 1)),
    )
    in_maps = []
    for r in range(NCORES):
        m = dict(base)
        m["img_t"] = _f16(img[NL * r: NL * (r + 1)].reshape(NS, C).T)
        m["wp_aug"] = _f16(np.concatenate(
            [wp_w[VS * r: VS * (r + 1)].T, wp_b[None, VS * r: VS * (r + 1)]], axis=0))
        in_maps.append(m)
    return in_maps


def build(nc, n_steps=T):
    mm = nc.tensor.matmul
    d_img = nc.dram_tensor("img_t", [C, NS], F16, kind="ExternalInput").ap()
    d_convw = nc.dram_tensor("conv_wT_aug", [C + 1, M], F16, kind="ExternalInput").ap()
    d_fcgw = nc.dram_tensor("fcg_wT", [C, E], F16, kind="ExternalInput").ap()
    d_fcgb = nc.dram_tensor("fcg_b", [E, 1], F32, kind="ExternalInput").ap()
    d_emb = nc.dram_tensor("emb", [V, E], F32, kind="ExternalInput").ap()
    d_seq = nc.dram_tensor("seq_idx", [T * N, 1], I32, kind="ExternalInput").ap()
    d_w0e = nc.dram_tensor("W0e", [E + 1, G2], F16, kind="ExternalInput").ap()
    d_w0c = nc.dram_tensor("W0c", [M, G2], F16, kind="ExternalInput").ap()
    d_w0h = nc.dram_tensor("W0h", [E, G2], F16, kind="ExternalInput").ap()
    d_w1x = nc.dram_tensor("W1x", [2 * E + 1, G2], F16, kind="ExternalInput").ap()
    d_w1h = nc.dram_tensor("W1h", [E, G2], F16, kind="ExternalInput").ap()
    d_lin = nc.dram_tensor("lin_aug", [2 * E + 1 + M, M], F16, kind="ExternalInput").ap()
    d_wp = nc.dram_tensor("wp_aug", [M + 1, VS], F16, kind="ExternalInput").ap()
    d_out = nc.dram_tensor("out_logits", [T, N, VS], F32, kind="ExternalOutput").ap()

    d_lraw = nc.dram_tensor("logits_raw", [T, N, LRAW_W], F16).ap()
    d_agm_in = nc.dram_tensor("agm_in", [E * NL * M], F8).ap()
    d_agm_out = nc.dram_tensor("agm_out", [NCORES * E * NL * M], F8,
                               addr_space="Shared").ap()
    d_agg_in = nc.dram_tensor("agg_in", [E * NL], F16).ap()
    d_agg_out = nc.dram_tensor("agg_out", [NCORES * E * NL], F16,
                               addr_space="Shared").ap()
    d_s_in = nc.dram_tensor("s_in", [N * T], F32).ap()
    d_s_out = nc.dram_tensor("s_out", [N * T], F32, addr_space="Shared").ap()

    with tile.TileContext(nc) as tc:
        wpool = tc.alloc_tile_pool(name="wpool", bufs=1)
        state = tc.alloc_tile_pool(name="state", bufs=1)
        work = tc.alloc_tile_pool(name="work", bufs=1)
        tiny = tc.alloc_tile_pool(name="tiny", bufs=1)
        psum = tc.alloc_tile_pool(name="psum", bufs=4, space="PSUM")
        initp = tc.alloc_tile_pool(name="initp", bufs=1)

        # ---------- persistent weights ----------
        def load_w(name, dram, blocks, width):
            t = wpool.tile([128, len(blocks), width], F16, name=name)
            for b, (r0, sz) in enumerate(blocks):
                nc.sync.dma_start(out=t[:sz, b, :], in_=dram[r0:r0 + sz, :])
            return t

        B128 = lambda rows: [(i, min(128, rows - i)) for i in range(0, rows, 128)]
        w0e = load_w("w0e", d_w0e, [(0, 128), (128, 69)], G2)
        w0c = load_w("w0c", d_w0c, B128(M), G2)
        w0h = load_w("w0h", d_w0h, [(0, 128), (128, 68)], G2)
        w1x = load_w("w1x", d_w1x, [(0, 128), (128, 68), (196, 128), (324, 69)], G2)
        w1h = load_w("w1h", d_w1h, [(0, 128), (128, 68)], G2)
        lin_sb = load_w("lin_sb", d_lin,
                        [(0, 128), (128, 68), (196, 128), (324, 69),
                         (393, 128), (521, 128), (649, 128), (777, 128)], M)
        wp_sb = load_w("wp_sb", d_wp, B128(M) + [(512, 1)], VS)

        idn16 = wpool.tile([128, 128], F16, name="idn16")
        make_identity(nc, idn16)
        idn32 = wpool.tile([128, 128], F32, name="idn32")
        make_identity(nc, idn32)
        ones1 = wpool.tile([1, T * N], F16, name="ones1")
        nc.vector.memset(ones1, 1.0)

        e_allT = wpool.tile([128, 2, T * N], F16, name="e_allT")
        g_allT = wpool.tile([128, 2, N], F16, name="g_allT")

        # ---------- recurrent state ----------
        h0T = state.tile([128, 4, N], F16, name="h0T")
        h1T = state.tile([128, 4, N], F16, name="h1T")
        ctxT = state.tile([128, 4, N], F16, name="ctxT")
        aT = state.tile([128, 5, N], F16, name="aT")
        Ct0 = state.tile([N, 2, E], F32, name="Ct0")
        Ct1 = state.tile([N, 2, E], F32, name="Ct1")
        sAll = state.tile([N, T], F32, name="sAll")
        for t_ in (ctxT, aT, Ct0, Ct1):
            nc.vector.memset(t_, 0.0)
        for t_ in (h0T, h1T):
            nc.vector.memset(t_[:, 0:3, :], 0.0)
            nc.vector.memset(t_[0:68, 3, :], 0.0)
        nc.gpsimd.dma_start(out=h0T[68:69, 3, :], in_=ones1[:, :N])
        nc.gpsimd.dma_start(out=h1T[68:69, 3, :], in_=ones1[:, :N])
        nc.vector.memset(aT[0:1, 4, :], 1.0)

        # ================= INIT =================
        img_sb = initp.tile([128, 16, NS], F16, name="img_sb")
        for kc in range(16):
            nc.sync.dma_start(out=img_sb[:, kc, :],
                              in_=d_img[128 * kc:128 * (kc + 1), :])
        convw_sb = initp.tile([128, 17, M], F16, name="convw_sb")
        for b, (r0, sz) in enumerate(B128(C) + [(C, 1)]):
            nc.sync.dma_start(out=convw_sb[:sz, b, :], in_=d_convw[r0:r0 + sz, :])
        fcgw_sb = initp.tile([128, 16, E], F16, name="fcgw_sb")
        for b, (r0, sz) in enumerate(B128(C)):
            nc.sync.dma_start(out=fcgw_sb[:sz, b, :], in_=d_fcgw[r0:r0 + sz, :])
        fcgb_sb = initp.tile([128, 2, 1], F32, name="fcgb_sb")
        nc.sync.dma_start(out=fcgb_sb[:, 0, :], in_=d_fcgb[0:128, :])
        nc.sync.dma_start(out=fcgb_sb[:68, 1, :], in_=d_fcgb[128:196, :])

        # --- conv -> mapped shard -> DRAM (rank layout (s, n_local, m))
        for mt0, msz in B128(NS):
            cps = psum.tile([128, 2, 512], F32, name="cps", tag="pair")
            for kc in range(16):
                mm(out=cps[:msz, 0, :], lhsT=img_sb[:, kc, mt0:mt0 + msz],
                   rhs=convw_sb[:, kc, :], start=(kc == 0), stop=False)
            mm(out=cps[:msz, 0, :], lhsT=ones1[:, :msz], rhs=convw_sb[0:1, 16, :],
               start=False, stop=True)
            ccast = initp.tile([128, M], F8, name="ccast", bufs=3)
            nc.vector.tensor_copy(out=ccast[:msz, :], in_=cps[:msz, 0, :])
            # scatter rows (n s) -> (s*8 + n)*512, per-n affine segments
            j = 0
            while j < msz:
                gi = mt0 + j
                n_, s_ = gi // E, gi % E
                take = min(msz - j, E - s_)
                dst = bass.AP(tensor=d_agm_in.tensor,
                              offset=(s_ * NL + n_) * M,
                              ap=[[NL * M, take], [1, M]])
                nc.sync.dma_start(out=dst, in_=ccast[j:j + take, :])
                j += take

        # --- g = mean_s(img) @ fcg_w.T + fcg_b  (via P = fcg_w @ img_t, reduce s)
        for mt, (m0, msz) in enumerate([(0, 128), (128, 68)]):
            p01 = psum.tile([128, 2, 512], F32, name="p01", tag="pair")
            p23 = psum.tile([128, 2, 512], F32, name="p23", tag="pair")
            tgt = [(p01, 0), (p01, 1), (p23, 0), (p23, 1)]
            for kc in range(16):
                for nt in range(4):
                    pt, sl = tgt[nt]
                    mm(out=pt[:msz, sl, :GNT], lhsT=fcgw_sb[:, kc, m0:m0 + msz],
                       rhs=img_sb[:, kc, GNT * nt:GNT * (nt + 1)],
                       start=(kc == 0), stop=(kc == 15))
            gpre = initp.tile([128, 8], F32, name="gpre", bufs=2)
            for half, pt in enumerate((p01, p23)):
                src = pt[:msz, :, :GNT].rearrange("p a (b s) -> p a b s", s=E)
                nc.vector.tensor_reduce(out=gpre[:msz, 4 * half:4 * half + 4],
                                        in_=src, axis=mybir.AxisListType.X,
                                        op=OP.add)
            g16 = initp.tile([128, 8], F16, name="g16", bufs=2)
            nc.scalar.activation(out=g16[:msz, :], in_=gpre[:msz, :], func=AF.Identity,
                                 bias=fcgb_sb[:msz, mt, :], scale=1.0 / E)
            dst = bass.AP(tensor=d_agg_in.tensor, offset=m0 * NL,
                          ap=[[NL, msz], [1, NL]])
            nc.sync.dma_start(out=dst, in_=g16[:msz, :])

        # --- AllGathers
        nc.gpsimd.collective_compute("AllGather", OP.bypass, replica_groups=RG,
                                     ins=[d_agm_in[:]], outs=[d_agm_out[:]])
        nc.gpsimd.collective_compute("AllGather", OP.bypass, replica_groups=RG,
                                     ins=[d_agg_in[:]], outs=[d_agg_out[:]])

        # --- embedding gather + transpose
        seq_sb = initp.tile([128, 12], I32, name="seq_sb")
        nc.sync.dma_start(out=seq_sb,
                          in_=bass.AP(tensor=d_seq.tensor, offset=0,
                                      ap=[[1, 128], [128, 12]]))
        e_all = initp.tile([128, 12, E], F32, name="e_all")
        for b in range(12):
            nc.gpsimd.indirect_dma_start(
                out=e_all[:, b, :], out_offset=None, in_=d_emb[:],
                in_offset=bass.IndirectOffsetOnAxis(ap=seq_sb[:, b:b + 1], axis=0))
        for b in range(12):
            etp = psum.tile([128, 2, 128], F32, name="etp", tag="pair")
            nc.tensor.transpose(out=etp[:, 0, :], in_=e_all[:, b, 0:128], identity=idn32)
            nc.tensor.transpose(out=etp[:68, 1, :], in_=e_all[:, b, 128:196],
                                identity=idn32)
            nc.vector.tensor_copy(out=e_allT[:, 0, 128 * b:128 * (b + 1)],
                                  in_=etp[:, 0, :])
            nc.vector.tensor_copy(out=e_allT[:68, 1, 128 * b:128 * (b + 1)],
                                  in_=etp[:68, 1, :])
        nc.gpsimd.dma_start(out=e_allT[68:69, 1, :], in_=ones1[:, :T * N])

        initp.release()

        mappool = tc.alloc_tile_pool(name="mappool", bufs=1)
        mapped = mappool.tile([128, 2, N, M], F8, name="mapped")
        for cchunk, (s0, scnt) in enumerate([(0, 128), (128, 68)]):
            for r in range(NCORES):
                src = bass.AP(tensor=d_agm_out.tensor,
                              offset=(r * E + s0) * NL * M,
                              ap=[[NL * M, scnt], [M, NL], [1, M]])
                nc.sync.dma_start(out=mapped[:scnt, cchunk, NL * r:NL * (r + 1), :],
                                  in_=src)
        for half, (e0, ecnt) in enumerate([(0, 128), (128, 68)]):
            src = bass.AP(tensor=d_agg_out.tensor, offset=e0 * NL,
                          ap=[[NL, ecnt], [E * NL, NCORES], [1, NL]])
            nc.sync.dma_start(out=g_allT[:ecnt, half, :], in_=src)

        # ---------- shared step machinery ----------
        def ctx_update(lhsT_tile, blkA, blkB, Asz=128, Bsz=68):
            """ctx_raw[n,:] = mapped[n] @ col_n(lhsT); l2norm -> ctxT.

            Row n = 8p + 2j + s runs on col-group j, psum-tile p, slot s, so
            the sparse psum rows (partitions 0/32/64/96) re-pack densely with
            one affine SBUF->SBUF DMA per tile (DMA cannot read PSUM; DVE/ACT
            evacuate partition-preserving first).
            """
            ctx_raw = work.tile([N, M], F16, name="ctx_raw", tag="ctx_raw")
            for p in range(8):
                mv = psum.tile([128, 2, 512], F32, name="mv", tag="pair")
                for s in range(2):
                    for j in range(4):
                        n_ = 8 * p + 2 * j + s
                        for c, (blk, cnt) in enumerate(((blkA, Asz), (blkB, Bsz))):
                            mm(out=mv[32 * j:32 * j + 32, s, :],
                               lhsT=lhsT_tile[:cnt, blk, n_:n_ + 1].to_broadcast(
                                   [cnt, 32]),
                               rhs=mapped[:cnt, c, n_, :],
                               start=(c == 0), stop=(c == 1),
                               tile_position=(0, 32 * j))
                sp = work.tile([128, 2, 512], F16, name="sp", tag="sp", bufs=2)
                if p % 2 == 0:
                    nc.vector.tensor_copy(out=sp, in_=mv)
                else:
                    nc.scalar.copy(out=sp, in_=mv)
                nc.sync.dma_start(out=ctx_raw[8 * p:8 * p + 8, :],
                                  in_=sp[0:128:32, :, :])
            sq = work.tile([N, M], F16, name="sq", tag="sq")
            q = tiny.tile([N, 1], F32, name="q", tag="q")
            nc.vector.scalar_tensor_tensor(out=sq, in0=ctx_raw, scalar=0.0,
                                           in1=ctx_raw, op0=OP.add, op1=OP.mult,
                                           accum_out=q)
            # rsqrt via magic-constant + 2 Newton iterations
            yi = tiny.tile([N, 1], I32, name="yi", tag="yi")
            nc.vector.tensor_scalar(out=yi, in0=q.bitcast(I32), scalar1=1,
                                    scalar2=None, op0=OP.logical_shift_right)
            nc.vector.tensor_scalar(out=yi, in0=yi, scalar1=0x5f375a86,
                                    scalar2=-1, op0=OP.subtract, op1=OP.mult)
            y = yi.bitcast(F32)
            t1 = tiny.tile([N, 1], F32, name="t1", tag="t1")
            for _ in range(2):
                nc.vector.tensor_tensor(out=t1, in0=y, in1=y, op=OP.mult)
                nc.vector.tensor_tensor(out=t1, in0=t1, in1=q, op=OP.mult)
                nc.vector.tensor_scalar(out=t1, in0=t1, scalar1=-0.5, scalar2=1.5,
                                        op0=OP.mult, op1=OP.add)
                nc.vector.tensor_tensor(out=y, in0=y, in1=t1, op=OP.mult)
            ctx16 = work.tile([N, M], F16, name="ctx16", tag="ctx16")
            nc.vector.tensor_scalar(out=ctx16, in0=ctx_raw, scalar1=y,
                                    scalar2=None, op0=OP.mult)
            tpc = psum.tile([128, 4, N], F16, name="tpc", tag="pair")
            for b in range(4):
                nc.tensor.transpose(out=tpc[:, b, :], in_=ctx16[:, 128 * b:128 * (b + 1)],
                                    identity=idn16[0:N, 0:N])
                nc.vector.tensor_copy(out=ctxT[:, b, :], in_=tpc[:, b, :])

        def lstm_layer(t, layer):
            """Emit gate matmuls + cell math for one layer; returns nothing."""
            if layer == 0:
                wh, hT, Ct = w0h, h0T, Ct0
            else:
                wh, hT, Ct = w1h, h1T, Ct1
            xT = h0T  # layer-1 input
            dps = []
            for d in range(2):
                ps = psum.tile([64, 2, 512], F32, name=f"g{layer}d{d}", tag="pair")
                dps.append(ps)
                for sub in range(2):
                    col = d * 784 + sub * GNT
                    out = ps[:, sub, :GNT]
                    seqm = []
                    if layer == 0:
                        t64 = t * N
                        seqm.append((e_allT[:, 0, t64:t64 + N], w0e[:, 0, col:col + GNT]))
                        seqm.append((e_allT[:69, 1, t64:t64 + N], w0e[:69, 1, col:col + GNT]))
                    else:
                        for b, (r0, sz) in enumerate(HBLK):
                            szx = sz + 1 if b == 3 else sz  # include ones row
                            seqm.append((xT[:szx, b, :], w1x[:szx, b, col:col + GNT]))
                    # h-part: dir d -> blocks 2d, 2d+1
                    for cb, (blk, cnt) in enumerate(((2 * d, 128), (2 * d + 1, 68))):
                        seqm.append((hT[:cnt, blk, :], wh[:cnt, cb, col:col + GNT]))
                    if layer == 0:
                        for k in range(4):
                            seqm.append((ctxT[:, k, :], w0c[:, k, col:col + GNT]))
                    last = len(seqm) - 1
                    for i, (lh, rh) in enumerate(seqm):
                        mm(out=out, lhsT=lh, rhs=rh, start=(i == 0), stop=(i == last))
            Tg = work.tile([N, 4, GNT], F16, name=f"T{layer}", tag=f"T{layer}")
            for d in range(2):
                nc.scalar.activation(out=Tg[:, 2 * d:2 * d + 2, :],
                                     in_=dps[d][:, :, :GNT], func=AF.Tanh)
            T_i = Tg[:, 0::2, 0:E]
            T_f = Tg[:, 0::2, E:2 * E]
            T_o = Tg[:, 1::2, 0:E]
            T_g = Tg[:, 1::2, E:2 * E]
            u = work.tile([N, 2, E], F32, name="u", tag="u")
            sf = work.tile([N, 2, E], F32, name="sf", tag="sf")
            nc.vector.scalar_tensor_tensor(out=u, in0=T_i, scalar=1.0, in1=T_g,
                                           op0=OP.add, op1=OP.mult)
            nc.vector.tensor_scalar(out=sf, in0=T_f, scalar1=0.5, scalar2=0.5,
                                    op0=OP.mult, op1=OP.add)
            nc.vector.tensor_tensor(out=sf, in0=sf, in1=Ct, op=OP.mult)
            nc.vector.tensor_tensor(out=Ct, in0=u, in1=sf, op=OP.add)
            Tc = work.tile([N, 2, E], F16, name=f"Tc{layer}", tag="Tc")
            nc.scalar.activation(out=Tc, in_=Ct, func=AF.Tanh, scale=0.5)
            hh = work.tile([N, 2 * E], F16, name=f"h{layer}_", tag=f"h{layer}_")
            hhv = hh.rearrange("p (a b) -> p a b", a=2)
            nc.vector.scalar_tensor_tensor(out=hhv, in0=T_o, scalar=1.0, in1=Tc,
                                           op0=OP.add, op1=OP.mult)
            # transposes -> hT blocks
            tph = psum.tile([128, 4, N], F16, name=f"tph{layer}", tag="pair")
            for b, (c0, w) in enumerate(HBLK):
                nc.tensor.transpose(out=tph[:w, b, :], in_=hh[:, c0:c0 + w],
                                    identity=idn16[0:N, 0:N])
                nc.vector.tensor_copy(out=hT[:w, b, :], in_=tph[:w, b, :])

        def lin_vocab(t):
            lps = psum.tile([64, 2, 512], F32, name="lps", tag="pair")
            seqm = []
            for b, (r0, sz) in enumerate(HBLK):
                szx = sz + 1 if b == 3 else sz
                seqm.append((h1T[:szx, b, :], lin_sb[:szx, b, :]))
            for k in range(4):
                seqm.append((ctxT[:, k, :], lin_sb[:, 4 + k, :]))
            for i, (lh, rh) in enumerate(seqm):
                mm(out=lps[:, 0, :], lhsT=lh, rhs=rh, start=(i == 0),
                   stop=(i == len(seqm) - 1))
            a16 = work.tile([N, M], F16, name="a16", tag="a16")
            lk = work.tile([N, M], F16, name="lk", tag="lk")
            # leaky_relu(x) = max(x, 0.01x), exact; one PSUM input per op
            nc.vector.tensor_scalar(out=lk, in0=lps[:, 0, :], scalar1=0.01,
                                    scalar2=None, op0=OP.mult)
            nc.vector.tensor_tensor(out=a16, in0=lps[:, 0, :], in1=lk, op=OP.max)
            tpa = psum.tile([128, 4, N], F16, name="tpa", tag="pair")
            for b in range(4):
                nc.tensor.transpose(out=tpa[:, b, :], in_=a16[:, 128 * b:128 * (b + 1)],
                                    identity=idn16[0:N, 0:N])
                nc.vector.tensor_copy(out=aT[:, b, :], in_=tpa[:, b, :])
            vpsA = psum.tile([64, 2, 512], F32, name="vpsA", tag="pair")
            vpsB = psum.tile([64, 2, 512], F32, name="vpsB", tag="pair")
            for nt, (v0, w) in enumerate(VOC_NT):
                out = vpsA[:, nt, :] if nt < 2 else vpsB[:, 0, :w]
                for k in range(5):
                    cnt = 128 if k < 4 else 1
                    mm(out=out, lhsT=aT[:cnt, k, :], rhs=wp_sb[:cnt, k, v0:v0 + w],
                       start=(k == 0), stop=(k == 4))
            xraw = work.tile([N, LRAW_W], F16, name="xraw", tag="xraw", bufs=2)
            xv = xraw.rearrange("p (a b) -> p a b", a=3)
            nc.vector.tensor_copy(out=xv[:, 0:2, :], in_=vpsA)
            nc.vector.tensor_copy(out=xv[:, 2, :476], in_=vpsB[:, 0, :476])
            nc.sync.dma_start(out=d_lraw[t][:, :1500], in_=xraw[:, :1500])
            dump = work.tile([N, LRAW_W], F16, name="dump", tag="dump")
            s1 = tiny.tile([N, 1], F32, name="s1", tag="s1")
            s2 = tiny.tile([N, 1], F32, name="s2", tag="s2")
            dv = dump.rearrange("p (a b) -> p a b", a=3)
            nc.scalar.activation(out=dv[:, 0:2, :], in_=vpsA, func=AF.Exp,
                                 accum_out=s1)
            nc.scalar.activation(out=dv[:, 2, :476], in_=vpsB[:, 0, :476], func=AF.Exp,
                                 accum_out=s2)
            nc.vector.tensor_tensor(out=sAll[:, t:t + 1], in0=s1, in1=s2, op=OP.add)

        # ---------- initial context ----------
        ctx_update(g_allT, 0, 1)

        # ---------- steps ----------
        for t in range(n_steps):
            lstm_layer(t, 0)
            lstm_layer(t, 1)
            lin_vocab(t)
            ctx_update(h1T, 2, 3)

        mappool.release()

        # ---------- finale: AllReduce s, ln, subtract ----------
        nc.sync.dma_start(out=bass.AP(tensor=d_s_in.tensor, offset=0,
                                      ap=[[T, N], [1, T]]), in_=sAll)
        nc.gpsimd.collective_compute("AllReduce", OP.add, replica_groups=RG,
                                     ins=[d_s_in[:]], outs=[d_s_out[:]])
        finp = tc.alloc_tile_pool(name="finp", bufs=3)
        sg = state.tile([N, T], F32, name="sg")
        nc.sync.dma_start(out=sg, in_=bass.AP(tensor=d_s_out.tensor, offset=0,
                                              ap=[[T, N], [1, T]]))
        lns = state.tile([N, T], F32, name="lns")
        nc.scalar.activation(out=lns, in_=sg, func=AF.Ln)
        for t in range(T):
            xst = finp.tile([N, LRAW_W], F16, name="xst", tag="xst")
            nc.sync.dma_start(out=xst[:, :1500], in_=d_lraw[t][:, :1500])
            ot = finp.tile([N, VS], F32, name="ot", tag="ot")
            nc.vector.tensor_scalar(out=ot, in0=xst[:, 0:VS], scalar1=lns[:, t:t + 1],
                                    scalar2=None, op0=OP.subtract)
            nc.sync.dma_start(out=d_out[t], in_=ot)
        finp.release()
        for p in (psum, tiny, work, state, wpool):
            p.release()
    return nc


_CACHED = {}


def _build_nc(n_steps=T):
    key = ("nc", n_steps)
    if key not in _CACHED:
        nc = bacc.Bacc("TRN2", target_bir_lowering=False, debug=False,
                       num_devices=NCORES)
        build(nc, n_steps)
        nc.compile()
        _CACHED[key] = nc
    return _CACHED[key]


def run(inputs, trace=False):
    nc = _build_nc()
    in_maps = prepare_inputs(inputs)
    res = run_bass_kernel_spmd(nc, in_maps, list(range(NCORES)), trace=trace)
    out = np.concatenate([res.results[r]["out_logits"] for r in range(NCORES)],
                         axis=2)
    return out.astype(np.float32), res


def kernel(**inputs):
    out, _ = run(inputs, trace=False)
    return out



# revision 2
# speedup vs baseline: 1.1764x; 1.1764x over previous
"""Trainium2 Bass kernel for nn_Caption (bidirectional-LSTM image captioner).

Distribution over 8 NeuronCores (zero per-step collectives):
  - Recurrent computation (both LSTM layers, lin, context attention) is
    REPLICATED on all cores with the full batch of 64: per-step gate matmuls
    are PE-streaming-bound (cost independent of batch <= 128), so replication
    is free and avoids per-step collectives (AllGather floor ~5us x 24 steps).
  - Vocab projection (12000) is sharded 8-way (1500 cols/core).
  - The 1x1 conv ("mapped") is sharded by batch (8 rows/core) and AllGathered
    once (fp16) at the start; every core holds the full mapped for the
    per-step context matvecs.
  - log_softmax: logits are tiny so no max-subtraction is needed; each core
    accumulates per-(t,n) sum of exp over its vocab slice; ONE AllReduce of
    (64,24) sums at the end; final pass writes x - ln(s_global).

Per-step schedule is software-pipelined so the PE never idles during the
cell-math / l2norm chains: lin+vocab of step t-1 are issued into step t's
gate phases, and gates0 of step t+1 is split into an (emb,h) partial
(issued during step t's ctx-norm tail) plus a ctx part once ctxT lands.

Layout: all matmuls are activation-stationary (lhsT = activations^T), so
activations are transposed each step via PE transposes.  Biases ride as
extra contraction rows against constant-1 rows in the transposed
activations.  sigma(x)=0.5*tanh(x/2)+0.5 with the 0.5 pre-scaled into the
i/f/o weight columns so one plain tanh covers all gates.  Cell state is kept
scaled (Ct=2c, h~=2h) with 0.5 folded into downstream weights; the
l2-normalized ctx is invariant to the h~ scaling.
"""

import sys
import numpy as np

for _p in ("/opt/trn_rl_repo",):
    if _p not in sys.path:
        sys.path.insert(0, _p)

import concourse.bass as bass
import concourse.tile as tile
from concourse import bacc
from concourse import mybir
from concourse.masks import make_identity
from concourse.bass_utils import run_bass_kernel_spmd

F16 = mybir.dt.float16
F8 = mybir.dt.float8e4
F32 = mybir.dt.float32
I32 = mybir.dt.int32
AF = mybir.ActivationFunctionType
OP = mybir.AluOpType

N = 64          # batch
T = 24          # steps
E = 196         # embedding/hidden size
M = 512         # context dim
C = 2048        # image channels
V = 12000       # vocab
NCORES = 8
VS = V // NCORES          # vocab slice per core
NL = N // NCORES          # batch rows per core (conv shard)
NS = NL * E               # conv rows per core (1568)
G2 = 2 * 4 * E            # gate cols, both dirs (1568)
RG = [list(range(NCORES))]
GNT = 392                 # gates N-tile
VOC_NT = [(0, 512), (512, 512), (1024, 476)]
LRAW_W = 1536             # padded row width of raw-logit staging

# h^T tiles are blocked {128, 68, 128, 68(+ones)} so fwd/bwd chunks align.
HBLK = [(0, 128), (128, 68), (196, 128), (324, 68)]


def _f16(x):
    return np.ascontiguousarray(x, dtype=np.float16)


def _f32(x):
    return np.ascontiguousarray(x, dtype=np.float32)


def prepare_inputs(inputs):
    img = _f32(np.asarray(inputs["input_image_feat"])).reshape(N, E, C)
    seq = np.ascontiguousarray(np.asarray(inputs["sequences"]).astype(np.int32))
    conv_w = _f32(inputs["conv_w"]); conv_b = _f32(inputs["conv_b"])
    fcg_w = _f32(inputs["fcg_w"]); fcg_b = _f32(inputs["fcg_b"])
    emb = _f32(inputs["emb"])
    w_ih0 = _f32(inputs["w_ih0"]); w_hh0 = _f32(inputs["w_hh0"]); b0 = _f32(inputs["b0"])
    w_ih1 = _f32(inputs["w_ih1"]); w_hh1 = _f32(inputs["w_hh1"]); b1 = _f32(inputs["b1"])
    lin_w = _f32(inputs["lin_w"]); lin_b = _f32(inputs["lin_b"])
    wp_w = _f32(inputs["wp_w"]); wp_b = _f32(inputs["wp_b"])

    # gate reorder [i f g o] -> [i f o g]; pre-scale i/f/o columns by 0.5
    perm = np.r_[0:E, E:2 * E, 3 * E:4 * E, 2 * E:3 * E]
    gsc = np.ones(4 * E, np.float32)
    gsc[: 3 * E] = 0.5

    def gmat(w):            # (784, in) -> (in, 784) permuted + scaled
        return w.T[:, perm] * gsc

    def gvec(b):
        return b[perm] * gsc

    W0 = np.concatenate([gmat(w_ih0[0]), gmat(w_ih0[1])], axis=1)        # (708,1568)
    b0r = np.concatenate([gvec(b0[0]), gvec(b0[1])])
    W0e = _f16(np.concatenate([W0[:E], b0r[None]], axis=0))              # (197,1568)
    W0c = _f16(W0[E:E + M])                                              # (512,1568)
    W0h = _f16(0.5 * np.concatenate([gmat(w_hh0[0]), gmat(w_hh0[1])], 1))  # (196,1568)
    W1 = 0.5 * np.concatenate([gmat(w_ih1[0]), gmat(w_ih1[1])], axis=1)  # (392,1568)
    b1r = np.concatenate([gvec(b1[0]), gvec(b1[1])])
    W1x = _f16(np.concatenate([W1, b1r[None]], axis=0))                  # (393,1568)
    W1h = _f16(0.5 * np.concatenate([gmat(w_hh1[0]), gmat(w_hh1[1])], 1))  # (196,1568)
    lin_aug = _f16(np.concatenate(                                       # (905,512)
        [0.5 * lin_w.T[:2 * E], lin_b[None], lin_w.T[2 * E:]], axis=0))
    conv_wT_aug = _f16(np.concatenate([conv_w.T, conv_b[None]], axis=0))  # (2049,512)

    base = dict(
        W0e=W0e, W0c=W0c, W0h=W0h, W1x=W1x, W1h=W1h, lin_aug=lin_aug,
        conv_wT_aug=conv_wT_aug, fcg_wT=_f16(fcg_w.T),
        fcg_b=_f32(fcg_b.reshape(E, 1)), emb=emb,
        seq_idx=np.ascontiguousarray(seq.reshape(T * N, 1)),
    )
    in_maps = []
    for r in range(NCORES):
        m = dict(base)
        m["img_t"] = _f16(img[NL * r: NL * (r + 1)].reshape(NS, C).T)
        m["wp_aug"] = _f16(np.concatenate(
            [wp_w[VS * r: VS * (r + 1)].T, wp_b[None, VS * r: VS * (r + 1)]], axis=0))
        in_maps.append(m)
    return in_maps


def build(nc, n_steps=T):
    mm = nc.tensor.matmul
    d_img = nc.dram_tensor("img_t", [C, NS], F16, kind="ExternalInput").ap()
    d_convw = nc.dram_tensor("conv_wT_aug", [C + 1, M], F16, kind="ExternalInput").ap()
    d_fcgw = nc.dram_tensor("fcg_wT", [C, E], F16, kind="ExternalInput").ap()
    d_fcgb = nc.dram_tensor("fcg_b", [E, 1], F32, kind="ExternalInput").ap()
    d_emb = nc.dram_tensor("emb", [V, E], F32, kind="ExternalInput").ap()
    d_seq = nc.dram_tensor("seq_idx", [T * N, 1], I32, kind="ExternalInput").ap()
    d_w0e = nc.dram_tensor("W0e", [E + 1, G2], F16, kind="ExternalInput").ap()
    d_w0c = nc.dram_tensor("W0c", [M, G2], F16, kind="ExternalInput").ap()
    d_w0h = nc.dram_tensor("W0h", [E, G2], F16, kind="ExternalInput").ap()
    d_w1x = nc.dram_tensor("W1x", [2 * E + 1, G2], F16, kind="ExternalInput").ap()
    d_w1h = nc.dram_tensor("W1h", [E, G2], F16, kind="ExternalInput").ap()
    d_lin = nc.dram_tensor("lin_aug", [2 * E + 1 + M, M], F16, kind="ExternalInput").ap()
    d_wp = nc.dram_tensor("wp_aug", [M + 1, VS], F16, kind="ExternalInput").ap()
    d_out = nc.dram_tensor("out_logits", [T, N, VS], F32, kind="ExternalOutput").ap()

    d_lraw = nc.dram_tensor("logits_raw", [T, N, LRAW_W], F16).ap()
    d_agm_in = nc.dram_tensor("agm_in", [E * NL * M], F8).ap()
    d_agm_out = nc.dram_tensor("agm_out", [NCORES * E * NL * M], F8,
                               addr_space="Shared").ap()
    d_agg_in = nc.dram_tensor("agg_in", [E * NL], F16).ap()
    d_agg_out = nc.dram_tensor("agg_out", [NCORES * E * NL], F16,
                               addr_space="Shared").ap()
    d_s_in = nc.dram_tensor("s_in", [N * T], F32).ap()
    d_s_out = nc.dram_tensor("s_out", [N * T], F32, addr_space="Shared").ap()

    with tile.TileContext(nc) as tc:
        wpool = tc.alloc_tile_pool(name="wpool", bufs=1)
        state = tc.alloc_tile_pool(name="state", bufs=1)
        work = tc.alloc_tile_pool(name="work", bufs=1)
        tiny = tc.alloc_tile_pool(name="tiny", bufs=1)
        psum = tc.alloc_tile_pool(name="psum", bufs=1, space="PSUM")
        initp = tc.alloc_tile_pool(name="initp", bufs=1)

        # psum rings: "g" = 4 x [64,512] f32 (2KB) gate/lin tiles;
        #             "mv" = 2 x 4KB shared by ctx matvec / vocab / transposes.
        def gtile(name):
            return psum.tile([N, 512], F32, name=name, tag="g", bufs=4)

        def mvtile(shape, dt, name):
            return psum.tile(shape, dt, name=name, tag="mv", bufs=2)

        # ---------- persistent weights ----------
        def load_w(name, dram, blocks, width):
            t = wpool.tile([128, len(blocks), width], F16, name=name)
            for b, (r0, sz) in enumerate(blocks):
                nc.sync.dma_start(out=t[:sz, b, :], in_=dram[r0:r0 + sz, :])
            return t

        B128 = lambda rows: [(i, min(128, rows - i)) for i in range(0, rows, 128)]
        w0e = load_w("w0e", d_w0e, [(0, 128), (128, 69)], G2)
        w0c = load_w("w0c", d_w0c, B128(M), G2)
        w0h = load_w("w0h", d_w0h, [(0, 128), (128, 68)], G2)
        w1x = load_w("w1x", d_w1x, [(0, 128), (128, 68), (196, 128), (324, 69)], G2)
        w1h = load_w("w1h", d_w1h, [(0, 128), (128, 68)], G2)
        lin_sb = load_w("lin_sb", d_lin,
                        [(0, 128), (128, 68), (196, 128), (324, 69),
                         (393, 128), (521, 128), (649, 128), (777, 128)], M)
        wp_sb = load_w("wp_sb", d_wp, B128(M) + [(512, 1)], VS)

        idn16 = wpool.tile([128, 128], F16, name="idn16")
        make_identity(nc, idn16)
        idn32 = wpool.tile([128, 128], F32, name="idn32")
        make_identity(nc, idn32)
        ones1 = wpool.tile([1, T * N], F16, name="ones1")
        nc.vector.memset(ones1, 1.0)

        e_allT = wpool.tile([128, 2, T * N], F16, name="e_allT")
        g_allT = wpool.tile([128, 2, N], F16, name="g_allT")

        # ---------- recurrent state ----------
        h0T = state.tile([128, 4, N], F16, name="h0T")
        h1T = state.tile([128, 4, N], F16, name="h1T")
        ctxT = state.tile([128, 2, 4, N], F16, name="ctxT")  # parity-double-buffered
        aT = state.tile([128, 5, N], F16, name="aT")
        Ct0 = state.tile([N, 2, E], F16, name="Ct0")
        Ct1 = state.tile([N, 2, E], F16, name="Ct1")
        sAll = state.tile([N, T], F32, name="sAll")
        for t_ in (ctxT, aT, Ct0, Ct1):
            nc.vector.memset(t_, 0.0)
        for t_ in (h0T, h1T):
            nc.vector.memset(t_[:, 0:3, :], 0.0)
            nc.vector.memset(t_[0:68, 3, :], 0.0)
        nc.gpsimd.dma_start(out=h0T[68:69, 3, :], in_=ones1[:, :N])
        nc.gpsimd.dma_start(out=h1T[68:69, 3, :], in_=ones1[:, :N])
        nc.vector.memset(aT[0:1, 4, :], 1.0)

        # ================= INIT =================
        img_sb = initp.tile([128, 16, NS], F16, name="img_sb")
        for kc in range(16):
            nc.sync.dma_start(out=img_sb[:, kc, :],
                              in_=d_img[128 * kc:128 * (kc + 1), :])
        convw_sb = initp.tile([128, 17, M], F16, name="convw_sb")
        for b, (r0, sz) in enumerate(B128(C) + [(C, 1)]):
            nc.sync.dma_start(out=convw_sb[:sz, b, :], in_=d_convw[r0:r0 + sz, :])
        fcgw_sb = initp.tile([128, 16, E], F16, name="fcgw_sb")
        for b, (r0, sz) in enumerate(B128(C)):
            nc.sync.dma_start(out=fcgw_sb[:sz, b, :], in_=d_fcgw[r0:r0 + sz, :])
        fcgb_sb = initp.tile([128, 2, 1], F32, name="fcgb_sb")
        nc.sync.dma_start(out=fcgb_sb[:, 0, :], in_=d_fcgb[0:128, :])
        nc.sync.dma_start(out=fcgb_sb[:68, 1, :], in_=d_fcgb[128:196, :])

        # --- conv -> mapped shard -> DRAM (rank layout (s, n_local, m))
        for mt0, msz in B128(NS):
            cps = mvtile([128, 2, 512], F32, "cps")
            for kc in range(16):
                mm(out=cps[:msz, 0, :], lhsT=img_sb[:, kc, mt0:mt0 + msz],
                   rhs=convw_sb[:, kc, :], start=(kc == 0), stop=False)
            mm(out=cps[:msz, 0, :], lhsT=ones1[:, :msz], rhs=convw_sb[0:1, 16, :],
               start=False, stop=True)
            ccast = initp.tile([128, M], F8, name="ccast", bufs=3)
            nc.vector.tensor_copy(out=ccast[:msz, :], in_=cps[:msz, 0, :])
            # scatter rows (n s) -> (s*8 + n)*512, per-n affine segments
            j = 0
            while j < msz:
                gi = mt0 + j
                n_, s_ = gi // E, gi % E
                take = min(msz - j, E - s_)
                dst = bass.AP(tensor=d_agm_in.tensor,
                              offset=(s_ * NL + n_) * M,
                              ap=[[NL * M, take], [1, M]])
                nc.sync.dma_start(out=dst, in_=ccast[j:j + take, :])
                j += take

        # --- g = mean_s(img) @ fcg_w.T + fcg_b  (via P = fcg_w @ img_t, reduce s)
        for mt, (m0, msz) in enumerate([(0, 128), (128, 68)]):
            p01 = mvtile([128, 2, 512], F32, "p01")
            p23 = mvtile([128, 2, 512], F32, "p23")
            tgt = [(p01, 0), (p01, 1), (p23, 0), (p23, 1)]
            for kc in range(16):
                for nt in range(4):
                    pt, sl = tgt[nt]
                    mm(out=pt[:msz, sl, :GNT], lhsT=fcgw_sb[:, kc, m0:m0 + msz],
                       rhs=img_sb[:, kc, GNT * nt:GNT * (nt + 1)],
                       start=(kc == 0), stop=(kc == 15))
            gpre = initp.tile([128, 8], F32, name="gpre", bufs=2)
            for half, pt in enumerate((p01, p23)):
                src = pt[:msz, :, :GNT].rearrange("p a (b s) -> p a b s", s=E)
                nc.vector.tensor_reduce(out=gpre[:msz, 4 * half:4 * half + 4],
                                        in_=src, axis=mybir.AxisListType.X,
                                        op=OP.add)
            g16 = initp.tile([128, 8], F16, name="g16", bufs=2)
            nc.scalar.activation(out=g16[:msz, :], in_=gpre[:msz, :], func=AF.Identity,
                                 bias=fcgb_sb[:msz, mt, :], scale=1.0 / E)
            dst = bass.AP(tensor=d_agg_in.tensor, offset=m0 * NL,
                          ap=[[NL, msz], [1, NL]])
            nc.sync.dma_start(out=dst, in_=g16[:msz, :])

        # --- AllGathers
        nc.gpsimd.collective_compute("AllGather", OP.bypass, replica_groups=RG,
                                     ins=[d_agm_in[:]], outs=[d_agm_out[:]])
        nc.gpsimd.collective_compute("AllGather", OP.bypass, replica_groups=RG,
                                     ins=[d_agg_in[:]], outs=[d_agg_out[:]])

        # --- embedding gather + transpose
        seq_sb = initp.tile([128, 12], I32, name="seq_sb")
        nc.sync.dma_start(out=seq_sb,
                          in_=bass.AP(tensor=d_seq.tensor, offset=0,
                                      ap=[[1, 128], [128, 12]]))
        e_all = initp.tile([128, 12, E], F32, name="e_all")
        for b in range(12):
            nc.gpsimd.indirect_dma_start(
                out=e_all[:, b, :], out_offset=None, in_=d_emb[:],
                in_offset=bass.IndirectOffsetOnAxis(ap=seq_sb[:, b:b + 1], axis=0))
        for b in range(12):
            etp = mvtile([128, 2, 128], F32, "etp")
            nc.tensor.transpose(out=etp[:, 0, :], in_=e_all[:, b, 0:128], identity=idn32)
            nc.tensor.transpose(out=etp[:68, 1, :], in_=e_all[:, b, 128:196],
                                identity=idn32)
            nc.vector.tensor_copy(out=e_allT[:, 0, 128 * b:128 * (b + 1)],
                                  in_=etp[:, 0, :])
            nc.vector.tensor_copy(out=e_allT[:68, 1, 128 * b:128 * (b + 1)],
                                  in_=etp[:68, 1, :])
        nc.gpsimd.dma_start(out=e_allT[68:69, 1, :], in_=ones1[:, :T * N])

        initp.release()

        mappool = tc.alloc_tile_pool(name="mappool", bufs=1)
        mapped = mappool.tile([128, 2, N, M], F8, name="mapped")
        for cchunk, (s0, scnt) in enumerate([(0, 128), (128, 68)]):
            for r in range(NCORES):
                src = bass.AP(tensor=d_agm_out.tensor,
                              offset=(r * E + s0) * NL * M,
                              ap=[[NL * M, scnt], [M, NL], [1, M]])
                nc.sync.dma_start(out=mapped[:scnt, cchunk, NL * r:NL * (r + 1), :],
                                  in_=src)
        for half, (e0, ecnt) in enumerate([(0, 128), (128, 68)]):
            src = bass.AP(tensor=d_agg_out.tensor, offset=e0 * NL,
                          ap=[[NL, ecnt], [E * NL, NCORES], [1, NL]])
            nc.sync.dma_start(out=g_allT[:ecnt, half, :], in_=src)

        # ---------- step machinery ----------
        def ctx_mvs(lhsT_tile, blkA, blkB, Asz=128, Bsz=68):
            """ctx_raw[n,:] = mapped[n] @ col_n(lhsT); returns dense ctx_raw.

            Row n = 8p + 2j + s runs on col-group j, psum-tile p, slot s, so
            the sparse psum rows (partitions 0/32/64/96) re-pack densely with
            one affine SBUF->SBUF DMA per tile (DMA cannot read PSUM; DVE/ACT
            evacuate partition-preserving first).
            """
            ctx_raw = work.tile([N, M], F16, name="ctx_raw", tag="ctx_raw")
            for p in range(8):
                mv = mvtile([128, 2, 512], F32, "mv")
                for s in range(2):
                    for j in range(4):
                        n_ = 8 * p + 2 * j + s
                        for c, (blk, cnt) in enumerate(((blkA, Asz), (blkB, Bsz))):
                            mm(out=mv[32 * j:32 * j + 32, s, :],
                               lhsT=lhsT_tile[:cnt, blk, n_:n_ + 1].to_broadcast(
                                   [cnt, 32]),
                               rhs=mapped[:cnt, c, n_, :],
                               start=(c == 0), stop=(c == 1),
                               tile_position=(0, 32 * j))
                sp = work.tile([128, 2, 512], F16, name="sp", tag="sp", bufs=2)
                if p % 2 == 0:
                    nc.vector.tensor_copy(out=sp, in_=mv)
                else:
                    nc.scalar.copy(out=sp, in_=mv)
                nc.sync.dma_start(out=ctx_raw[8 * p:8 * p + 8, :],
                                  in_=sp[0:128:32, :, :])
            return ctx_raw

        def ctx_norm(ctx_raw, parity):
            """l2-normalize ctx_raw and transpose into ctxT[parity]."""
            sq = work.tile([N, M], F16, name="sq", tag="sq")
            q = tiny.tile([N, 1], F32, name="q", tag="q")
            nc.vector.scalar_tensor_tensor(out=sq, in0=ctx_raw, scalar=0.0,
                                           in1=ctx_raw, op0=OP.add, op1=OP.mult,
                                           accum_out=q)
            rq = tiny.tile([N, 1], F32, name="rq", tag="rq")
            nc.vector.reciprocal_approx_fast(out=rq, in_=q)
            rs = tiny.tile([N, 1], F32, name="rs", tag="rs")
            nc.scalar.activation(out=rs, in_=rq, func=AF.Sqrt)
            ctx16 = work.tile([N, M], F16, name="ctx16", tag="ctx16")
            nc.vector.tensor_scalar(out=ctx16, in0=ctx_raw, scalar1=rs,
                                    scalar2=None, op0=OP.mult)
            tpc = mvtile([128, 4, N], F16, "tpc")
            for b in range(4):
                nc.tensor.transpose(out=tpc[:, b, :], in_=ctx16[:, 128 * b:128 * (b + 1)],
                                    identity=idn16[0:N, 0:N])
                nc.vector.tensor_copy(out=ctxT[:, parity, b, :], in_=tpc[:, b, :])

        def l0_eh_mms(t):
            """Open gates0 psum tiles for step t; accumulate emb+h parts."""
            t64 = t * N
            tiles = []
            for d in range(2):
                for sub in range(2):
                    ps = gtile(f"g0d{d}s{sub}")
                    tiles.append(ps)
                    col = d * 784 + sub * GNT
                    out = ps[:, :GNT]
                    seqm = [(e_allT[:, 0, t64:t64 + N], w0e[:, 0, col:col + GNT]),
                            (e_allT[:69, 1, t64:t64 + N], w0e[:69, 1, col:col + GNT])]
                    for cb, (blk, cnt) in enumerate(((2 * d, 128), (2 * d + 1, 68))):
                        seqm.append((h0T[:cnt, blk, :], w0h[:cnt, cb, col:col + GNT]))
                    for i, (lh, rh) in enumerate(seqm):
                        mm(out=out, lhsT=lh, rhs=rh, start=(i == 0), stop=False)
            return tiles

        def l0_ctx_mms(tiles, parity):
            for d in range(2):
                for sub in range(2):
                    ps = tiles[2 * d + sub]
                    col = d * 784 + sub * GNT
                    out = ps[:, :GNT]
                    for k in range(4):
                        mm(out=out, lhsT=ctxT[:, parity, k, :],
                           rhs=w0c[:, k, col:col + GNT],
                           start=False, stop=(k == 3))

        def l1_mms(t):
            tiles = []
            for d in range(2):
                for sub in range(2):
                    ps = gtile(f"g1d{d}s{sub}")
                    tiles.append(ps)
                    col = d * 784 + sub * GNT
                    out = ps[:, :GNT]
                    seqm = []
                    for b, (r0, sz) in enumerate(HBLK):
                        szx = sz + 1 if b == 3 else sz  # include ones row
                        seqm.append((h0T[:szx, b, :], w1x[:szx, b, col:col + GNT]))
                    for cb, (blk, cnt) in enumerate(((2 * d, 128), (2 * d + 1, 68))):
                        seqm.append((h1T[:cnt, blk, :], w1h[:cnt, cb, col:col + GNT]))
                    last = len(seqm) - 1
                    for i, (lh, rh) in enumerate(seqm):
                        mm(out=out, lhsT=lh, rhs=rh, start=(i == 0), stop=(i == last))
            return tiles

        def gates_tanh(tiles, layer):
            Tg = work.tile([N, 4, GNT], F16, name=f"T{layer}", tag=f"T{layer}")
            for d in range(2):
                for sub in range(2):
                    nc.scalar.activation(out=Tg[:, 2 * d + sub, :],
                                         in_=tiles[2 * d + sub][:, :GNT],
                                         func=AF.Tanh)
            return Tg

        def cell_dve(Tg, layer):
            """fp16 cell math; returns hh [N, 2E] (h~ row layout)."""
            Ct = Ct0 if layer == 0 else Ct1
            T_i = Tg[:, 0::2, 0:E]
            T_f = Tg[:, 0::2, E:2 * E]
            T_o = Tg[:, 1::2, 0:E]
            T_g = Tg[:, 1::2, E:2 * E]
            u = work.tile([N, 2, E], F16, name="u", tag="u")
            sf = work.tile([N, 2, E], F16, name="sf", tag="sf")
            nc.vector.scalar_tensor_tensor(out=u, in0=T_i, scalar=1.0, in1=T_g,
                                           op0=OP.add, op1=OP.mult)
            nc.vector.tensor_scalar(out=sf, in0=T_f, scalar1=0.5, scalar2=0.5,
                                    op0=OP.mult, op1=OP.add)
            nc.vector.tensor_tensor(out=sf, in0=sf, in1=Ct, op=OP.mult)
            nc.vector.tensor_tensor(out=Ct, in0=u, in1=sf, op=OP.add)
            Tc = work.tile([N, 2, E], F16, name=f"Tc{layer}", tag="Tc")
            nc.scalar.activation(out=Tc, in_=Ct, func=AF.Tanh, scale=0.5)
            hh = work.tile([N, 2 * E], F16, name=f"h{layer}_", tag=f"h{layer}_")
            hhv = hh.rearrange("p (a b) -> p a b", a=2)
            nc.vector.scalar_tensor_tensor(out=hhv, in0=T_o, scalar=1.0, in1=Tc,
                                           op0=OP.add, op1=OP.mult)
            return hh

        def h_transpose(hh, hT, layer):
            tph = mvtile([128, 4, N], F16, f"tph{layer}")
            for b, (c0, w) in enumerate(HBLK):
                nc.tensor.transpose(out=tph[:w, b, :], in_=hh[:, c0:c0 + w],
                                    identity=idn16[0:N, 0:N])
                nc.vector.tensor_copy(out=hT[:w, b, :], in_=tph[:w, b, :])

        def lin_mms(parity):
            lps = gtile("lps")
            seqm = []
            for b, (r0, sz) in enumerate(HBLK):
                szx = sz + 1 if b == 3 else sz
                seqm.append((h1T[:szx, b, :], lin_sb[:szx, b, :]))
            for k in range(4):
                seqm.append((ctxT[:, parity, k, :], lin_sb[:, 4 + k, :]))
            for i, (lh, rh) in enumerate(seqm):
                mm(out=lps, lhsT=lh, rhs=rh, start=(i == 0),
                   stop=(i == len(seqm) - 1))
            return lps

        def lin_leaky(lps):
            a16 = work.tile([N, M], F16, name="a16", tag="a16")
            lk = work.tile([N, M], F16, name="lk", tag="lk")
            # leaky_relu(x) = max(x, 0.01x), exact; one PSUM input per op
            nc.vector.tensor_scalar(out=lk, in0=lps, scalar1=0.01,
                                    scalar2=None, op0=OP.mult)
            nc.vector.tensor_tensor(out=a16, in0=lps, in1=lk, op=OP.max)
            return a16

        def a_transpose(a16):
            tpa = mvtile([128, 4, N], F16, "tpa")
            for b in range(4):
                nc.tensor.transpose(out=tpa[:, b, :], in_=a16[:, 128 * b:128 * (b + 1)],
                                    identity=idn16[0:N, 0:N])
                nc.vector.tensor_copy(out=aT[:, b, :], in_=tpa[:, b, :])

        def vocab_A(t):
            """nt0+nt1 matmuls, exp+accum, raw-logit stash (first 1024 cols)."""
            vpsA = mvtile([N, 2, 512], F32, "vpsA")
            for nt in range(2):
                v0, w = VOC_NT[nt]
                for k in range(5):
                    cnt = 128 if k < 4 else 1
                    mm(out=vpsA[:, nt, :], lhsT=aT[:cnt, k, :],
                       rhs=wp_sb[:cnt, k, v0:v0 + w], start=(k == 0), stop=(k == 4))
            xraw = work.tile([N, LRAW_W], F16, name="xraw", tag="xraw", bufs=2)
            xv = xraw.rearrange("p (a b) -> p a b", a=3)
            dump = work.tile([N, LRAW_W], F16, name="dump", tag="dump", bufs=2)
            dv = dump.rearrange("p (a b) -> p a b", a=3)
            s1 = tiny.tile([N, 1], F32, name="s1", tag="s1")
            nc.scalar.activation(out=dv[:, 0:2, :], in_=vpsA, func=AF.Exp,
                                 accum_out=s1)
            nc.vector.tensor_copy(out=xv[:, 0:2, :], in_=vpsA)
            return xraw, dump, s1

        def vocab_B(t, xraw, dump, s1):
            vpsB = mvtile([N, 512], F32, "vpsB")
            v0, w = VOC_NT[2]
            for k in range(5):
                cnt = 128 if k < 4 else 1
                mm(out=vpsB[:, :w], lhsT=aT[:cnt, k, :],
                   rhs=wp_sb[:cnt, k, v0:v0 + w], start=(k == 0), stop=(k == 4))
            xv = xraw.rearrange("p (a b) -> p a b", a=3)
            dv = dump.rearrange("p (a b) -> p a b", a=3)
            s2 = tiny.tile([N, 1], F32, name="s2", tag="s2")
            nc.scalar.activation(out=dv[:, 2, :476], in_=vpsB[:, :476], func=AF.Exp,
                                 accum_out=s2)
            nc.vector.tensor_copy(out=xv[:, 2, :476], in_=vpsB[:, :476])
            nc.vector.tensor_tensor(out=sAll[:, t:t + 1], in0=s1, in1=s2, op=OP.add)
            nc.sync.dma_start(out=d_lraw[t][:, :1500], in_=xraw[:, :1500])

        # ---------- initial context (writes parity 1) ----------
        craw = ctx_mvs(g_allT, 0, 1)
        ctx_norm(craw, 1)

        # ---------- software-pipelined steps ----------
        g0_tiles = l0_eh_mms(0)
        prev = None  # (lps, ) state for step t-1 head
        for t in range(n_steps):
            l0_ctx_mms(g0_tiles, (t - 1) % 2)            # finish gates0(t)
            Tg0 = gates_tanh(g0_tiles, 0)
            if t > 0:
                lps = lin_mms((t - 2) % 2)               # lin(t-1), fills cell0 gap
            hh0 = cell_dve(Tg0, 0)
            h_transpose(hh0, h0T, 0)
            if t > 0:
                a16 = lin_leaky(lps)
            g1_tiles = l1_mms(t)
            Tg1 = gates_tanh(g1_tiles, 1)
            if t > 0:
                a_transpose(a16)
                vA = vocab_A(t - 1)                      # fills cell1 gap
            hh1 = cell_dve(Tg1, 1)
            h_transpose(hh1, h1T, 1)
            if t > 0:
                vocab_B(t - 1, *vA)
            craw = ctx_mvs(h1T, 2, 3)
            if t + 1 < n_steps:
                g0_tiles = l0_eh_mms(t + 1)              # fills ctx-norm tail
            ctx_norm(craw, t % 2)

        # trailing head for the last step
        lps = lin_mms((n_steps - 2) % 2)
        a16 = lin_leaky(lps)
        a_transpose(a16)
        vA = vocab_A(n_steps - 1)
        vocab_B(n_steps - 1, *vA)

        mappool.release()

        # ---------- finale: AllReduce s, ln, subtract ----------
        nc.sync.dma_start(out=bass.AP(tensor=d_s_in.tensor, offset=0,
                                      ap=[[T, N], [1, T]]), in_=sAll)
        nc.gpsimd.collective_compute("AllReduce", OP.add, replica_groups=RG,
                                     ins=[d_s_in[:]], outs=[d_s_out[:]])
        finp = tc.alloc_tile_pool(name="finp", bufs=3)
        sg = state.tile([N, T], F32, name="sg")
        nc.sync.dma_start(out=sg, in_=bass.AP(tensor=d_s_out.tensor, offset=0,
                                              ap=[[T, N], [1, T]]))
        lns = state.tile([N, T], F32, name="lns")
        nc.scalar.activation(out=lns, in_=sg, func=AF.Ln)
        nlns = state.tile([N, T], F32, name="nlns")
        nc.vector.tensor_scalar(out=nlns, in0=lns, scalar1=-1.0, scalar2=None,
                                op0=OP.mult)
        for t in range(T):
            xst = finp.tile([N, LRAW_W], F16, name="xst", tag="xst")
            nc.sync.dma_start(out=xst[:, :1500], in_=d_lraw[t][:, :1500])
            ot = finp.tile([N, VS], F32, name="ot", tag="ot")
            if t % 2 == 0:
                nc.vector.tensor_scalar(out=ot, in0=xst[:, 0:VS],
                                        scalar1=lns[:, t:t + 1],
                                        scalar2=None, op0=OP.subtract)
            else:
                nc.scalar.activation(out=ot, in_=xst[:, 0:VS], func=AF.Identity,
                                     bias=nlns[:, t:t + 1])
            nc.sync.dma_start(out=d_out[t], in_=ot)
        finp.release()
        for p in (psum, tiny, work, state, wpool):
            p.release()
    return nc


_CACHED = {}


def _build_nc(n_steps=T):
    key = ("nc", n_steps)
    if key not in _CACHED:
        nc = bacc.Bacc("TRN2", target_bir_lowering=False, debug=False,
                       num_devices=NCORES)
        build(nc, n_steps)
        nc.compile()
        _CACHED[key] = nc
    return _CACHED[key]


def run(inputs, trace=False):
    nc = _build_nc()
    in_maps = prepare_inputs(inputs)
    res = run_bass_kernel_spmd(nc, in_maps, list(range(NCORES)), trace=trace)
    out = np.concatenate([res.results[r]["out_logits"] for r in range(NCORES)],
                         axis=2)
    return out.astype(np.float32), res


def kernel(**inputs):
    out, _ = run(inputs, trace=False)
    return out


# revision 19
# speedup vs baseline: 1.2006x; 1.0206x over previous
"""Trainium2 Bass kernel for nn_Caption (bidirectional-LSTM image captioner).

Distribution over 8 NeuronCores (zero per-step collectives):
  - Recurrent computation (both LSTM layers, lin, context attention) is
    REPLICATED on all cores with the full batch of 64: per-step gate matmuls
    are PE-streaming-bound (cost independent of batch <= 128), so replication
    is free and avoids per-step collectives (AllGather floor ~5us x 24 steps).
  - Vocab projection (12000) is sharded 8-way (1500 cols/core).
  - The 1x1 conv ("mapped") is sharded by batch (8 rows/core) and AllGathered
    once (fp16) at the start; every core holds the full mapped for the
    per-step context matvecs.
  - log_softmax: logits are tiny so no max-subtraction is needed; each core
    accumulates per-(t,n) sum of exp over its vocab slice; ONE AllReduce of
    (64,24) sums at the end; final pass writes x - ln(s_global).

Per-step schedule is software-pipelined so the PE never idles during the
cell-math / l2norm chains: lin+vocab of step t-1 are issued into step t's
gate phases, and gates0 of step t+1 is split into an (emb,h) partial
(issued during step t's ctx-norm tail) plus a ctx part once ctxT lands.

Layout: all matmuls are activation-stationary (lhsT = activations^T), so
activations are transposed each step via PE transposes.  Biases ride as
extra contraction rows against constant-1 rows in the transposed
activations.  sigma(x)=0.5*tanh(x/2)+0.5 with the 0.5 pre-scaled into the
i/f/o weight columns so one plain tanh covers all gates.  Cell state is kept
scaled (Ct=2c, h~=2h) with 0.5 folded into downstream weights; the
l2-normalized ctx is invariant to the h~ scaling.
"""

import sys
import numpy as np
import ml_dtypes

for _p in ("/opt/trn_rl_repo",):
    if _p not in sys.path:
        sys.path.insert(0, _p)

import concourse.bass as bass
import concourse.tile as tile
from concourse import bacc
from concourse import mybir
from concourse.masks import make_identity
from concourse.bass_utils import run_bass_kernel_spmd

F16 = mybir.dt.float16
F8 = mybir.dt.float8e4
F32 = mybir.dt.float32
I32 = mybir.dt.int32
AF = mybir.ActivationFunctionType
OP = mybir.AluOpType

N = 64          # batch
T = 24          # steps
E = 196         # embedding/hidden size
M = 512         # context dim
C = 2048        # image channels
V = 12000       # vocab
NCORES = 8
VS = V // NCORES          # vocab slice per core
NL = N // NCORES          # batch rows per core (conv shard)
NS = NL * E               # conv rows per core (1568)
G2 = 2 * 4 * E            # gate cols, both dirs (1568)
RG = [list(range(NCORES))]
GNT = 392                 # gates N-tile
VOC_NT = [(0, 512), (512, 512), (1024, 476)]
LRAW_W = 1536             # padded row width of raw-logit staging

# h^T tiles are blocked {128, 68, 128, 68(+ones)} so fwd/bwd chunks align.
HBLK = [(0, 128), (128, 68), (196, 128), (324, 68)]


def _f16(x):
    return np.ascontiguousarray(x, dtype=np.float16)


def _f32(x):
    return np.ascontiguousarray(x, dtype=np.float32)


def prepare_inputs(inputs):
    img = _f32(np.asarray(inputs["input_image_feat"])).reshape(N, E, C)
    seq = np.ascontiguousarray(np.asarray(inputs["sequences"]).astype(np.int32))
    conv_w = _f32(inputs["conv_w"]); conv_b = _f32(inputs["conv_b"])
    fcg_w = _f32(inputs["fcg_w"]); fcg_b = _f32(inputs["fcg_b"])
    emb = _f32(inputs["emb"])
    w_ih0 = _f32(inputs["w_ih0"]); w_hh0 = _f32(inputs["w_hh0"]); b0 = _f32(inputs["b0"])
    w_ih1 = _f32(inputs["w_ih1"]); w_hh1 = _f32(inputs["w_hh1"]); b1 = _f32(inputs["b1"])
    lin_w = _f32(inputs["lin_w"]); lin_b = _f32(inputs["lin_b"])
    wp_w = _f32(inputs["wp_w"]); wp_b = _f32(inputs["wp_b"])

    # gate reorder [i f g o] -> [i f o g]; pre-scale i/f/o columns by 0.5
    perm = np.r_[0:E, E:2 * E, 3 * E:4 * E, 2 * E:3 * E]
    gsc = np.ones(4 * E, np.float32)
    gsc[: 3 * E] = 0.5

    def gmat(w):            # (784, in) -> (in, 784) permuted + scaled
        return w.T[:, perm] * gsc

    def gvec(b):
        return b[perm] * gsc

    W0 = np.concatenate([gmat(w_ih0[0]), gmat(w_ih0[1])], axis=1)        # (708,1568)
    b0r = np.concatenate([gvec(b0[0]), gvec(b0[1])])
    W0e = _f16(np.concatenate([W0[:E], b0r[None]], axis=0))              # (197,1568)
    W0c = _f16(W0[E:E + M])                                              # (512,1568)
    W0h = _f16(0.5 * np.concatenate([gmat(w_hh0[0]), gmat(w_hh0[1])], 1))  # (196,1568)
    W1 = 0.5 * np.concatenate([gmat(w_ih1[0]), gmat(w_ih1[1])], axis=1)  # (392,1568)
    b1r = np.concatenate([gvec(b1[0]), gvec(b1[1])])
    W1x = _f16(np.concatenate([W1, b1r[None]], axis=0))                  # (393,1568)
    W1h = _f16(0.5 * np.concatenate([gmat(w_hh1[0]), gmat(w_hh1[1])], 1))  # (196,1568)
    lin_aug = _f16(np.concatenate(                                       # (905,512)
        [0.5 * lin_w.T[:2 * E], lin_b[None], lin_w.T[2 * E:]], axis=0))
    conv_wT_aug = _f16(np.concatenate([conv_w.T, conv_b[None]], axis=0))  # (2049,512)

    base = dict(
        W0e=W0e, W0c=W0c, W0h=W0h, W1x=W1x, W1h=W1h, lin_aug=lin_aug,
        conv_wT_aug=conv_wT_aug, fcg_wT=_f16(fcg_w.T),
        fcg_b=_f32(fcg_b.reshape(E, 1)), emb=emb,
        seq_idx=np.ascontiguousarray(seq.reshape(T * N, 1)),
    )
    in_maps = []
    for r in range(NCORES):
        m = dict(base)
        m["img_t"] = np.ascontiguousarray(
            img[NL * r: NL * (r + 1)].reshape(NS, C).T.astype(ml_dtypes.float8_e4m3))
        m["wp_aug"] = _f16(np.concatenate(
            [wp_w[VS * r: VS * (r + 1)].T, wp_b[None, VS * r: VS * (r + 1)]], axis=0))
        in_maps.append(m)
    return in_maps


def build(nc, n_steps=T):
    mm = nc.tensor.matmul
    d_img = nc.dram_tensor("img_t", [C, NS], F8, kind="ExternalInput").ap()
    d_convw = nc.dram_tensor("conv_wT_aug", [C + 1, M], F16, kind="ExternalInput").ap()
    d_fcgw = nc.dram_tensor("fcg_wT", [C, E], F16, kind="ExternalInput").ap()
    d_fcgb = nc.dram_tensor("fcg_b", [E, 1], F32, kind="ExternalInput").ap()
    d_emb = nc.dram_tensor("emb", [V, E], F32, kind="ExternalInput").ap()
    d_seq = nc.dram_tensor("seq_idx", [T * N, 1], I32, kind="ExternalInput").ap()
    d_w0e = nc.dram_tensor("W0e", [E + 1, G2], F16, kind="ExternalInput").ap()
    d_w0c = nc.dram_tensor("W0c", [M, G2], F16, kind="ExternalInput").ap()
    d_w0h = nc.dram_tensor("W0h", [E, G2], F16, kind="ExternalInput").ap()
    d_w1x = nc.dram_tensor("W1x", [2 * E + 1, G2], F16, kind="ExternalInput").ap()
    d_w1h = nc.dram_tensor("W1h", [E, G2], F16, kind="ExternalInput").ap()
    d_lin = nc.dram_tensor("lin_aug", [2 * E + 1 + M, M], F16, kind="ExternalInput").ap()
    d_wp = nc.dram_tensor("wp_aug", [M + 1, VS], F16, kind="ExternalInput").ap()
    d_out = nc.dram_tensor("out_logits", [T, N, VS], F32, kind="ExternalOutput").ap()

    d_lraw = nc.dram_tensor("logits_raw", [T, N, LRAW_W], F16).ap()
    d_agm_in = nc.dram_tensor("agm_in", [E * NL * M], F8).ap()
    # mapped AllGather is chunked by s-range (98 rows each) so transfers
    # overlap the conv; each chunk's output is rank-major on its own buffer.
    SCH = 98
    d_agm_outs = [nc.dram_tensor(f"agm_out{k}", [NCORES * SCH * NL * M], F8,
                                 addr_space="Shared").ap() for k in range(2)]
    d_agg_in = nc.dram_tensor("agg_in", [E * NL], F16).ap()
    d_agg_out = nc.dram_tensor("agg_out", [NCORES * E * NL], F16,
                               addr_space="Shared").ap()
    d_s_in = nc.dram_tensor("s_in", [N * T], F32).ap()
    d_s_out = nc.dram_tensor("s_out", [N * T], F32, addr_space="Shared").ap()

    with tile.TileContext(nc) as tc:
        wpool = tc.alloc_tile_pool(name="wpool", bufs=1)
        state = tc.alloc_tile_pool(name="state", bufs=1)
        work = tc.alloc_tile_pool(name="work", bufs=1)
        tiny = tc.alloc_tile_pool(name="tiny", bufs=1)
        psum = tc.alloc_tile_pool(name="psum", bufs=1, space="PSUM")
        initp = tc.alloc_tile_pool(name="initp", bufs=1)

        # psum rings: "g" = 4 x [64,512] f32 (2KB) gate/lin tiles;
        #             "mv" = 2 x 4KB shared by ctx matvec / vocab / transposes.
        def gtile(name):
            return psum.tile([N, 512], F32, name=name, tag="g", bufs=4)

        def mvtile(shape, dt, name):
            return psum.tile(shape, dt, name=name, tag="mv", bufs=2)

        # ---------- persistent weights ----------
        def load_w(name, dram, blocks, width):
            t = wpool.tile([128, len(blocks), width], F16, name=name)
            for b, (r0, sz) in enumerate(blocks):
                nc.sync.dma_start(out=t[:sz, b, :], in_=dram[r0:r0 + sz, :])
            return t

        B128 = lambda rows: [(i, min(128, rows - i)) for i in range(0, rows, 128)]
        w0e = load_w("w0e", d_w0e, [(0, 128), (128, 69)], G2)
        w0c = load_w("w0c", d_w0c, B128(M), G2)
        w0h = load_w("w0h", d_w0h, [(0, 128), (128, 68)], G2)
        w1x = load_w("w1x", d_w1x, [(0, 128), (128, 68), (196, 128), (324, 69)], G2)
        w1h = load_w("w1h", d_w1h, [(0, 128), (128, 68)], G2)
        lin_sb = load_w("lin_sb", d_lin,
                        [(0, 128), (128, 68), (196, 128), (324, 69),
                         (393, 128), (521, 128), (649, 128), (777, 128)], M)
        wp_sb = load_w("wp_sb", d_wp, B128(M) + [(512, 1)], VS)

        idn16 = wpool.tile([128, 128], F16, name="idn16")
        make_identity(nc, idn16)
        idn32 = wpool.tile([128, 128], F32, name="idn32")
        make_identity(nc, idn32)
        ones1 = wpool.tile([1, T * N], F16, name="ones1")
        nc.vector.memset(ones1, 1.0)

        e_allT = wpool.tile([128, 2, T * N], F16, name="e_allT")
        g_allT = wpool.tile([128, 2, N], F16, name="g_allT")

        # ---------- recurrent state ----------
        h0T = state.tile([128, 4, N], F16, name="h0T")
        h1T = state.tile([128, 4, N], F16, name="h1T")
        ctxT = state.tile([128, 2, 4, N], F16, name="ctxT")  # parity-double-buffered
        aT = state.tile([128, 5, N], F16, name="aT")
        Ct0 = state.tile([N, 2, E], F16, name="Ct0")
        Ct1 = state.tile([N, 2, E], F16, name="Ct1")
        sAll = state.tile([N, T], F32, name="sAll")
        for t_ in (ctxT, aT, Ct0, Ct1):
            nc.vector.memset(t_, 0.0)
        for t_ in (h0T, h1T):
            nc.vector.memset(t_[:, 0:3, :], 0.0)
            nc.vector.memset(t_[0:68, 3, :], 0.0)
        nc.gpsimd.dma_start(out=h0T[68:69, 3, :], in_=ones1[:, :N])
        nc.gpsimd.dma_start(out=h1T[68:69, 3, :], in_=ones1[:, :N])
        nc.vector.memset(aT[0:1, 4, :], 1.0)

        # ================= INIT =================
        img_sb = initp.tile([128, 16, NS], F8, name="img_sb")
        for kc in range(16):
            nc.sync.dma_start(out=img_sb[:, kc, :],
                              in_=d_img[128 * kc:128 * (kc + 1), :])
        convw_sb = initp.tile([128, 17, M], F16, name="convw_sb")
        for b, (r0, sz) in enumerate(B128(C) + [(C, 1)]):
            nc.sync.dma_start(out=convw_sb[:sz, b, :], in_=d_convw[r0:r0 + sz, :])
        fcgw_sb = initp.tile([128, 16, E], F16, name="fcgw_sb")
        for b, (r0, sz) in enumerate(B128(C)):
            nc.sync.dma_start(out=fcgw_sb[:sz, b, :], in_=d_fcgw[r0:r0 + sz, :])
        fcgb_sb = initp.tile([128, 2, 1], F32, name="fcgb_sb")
        nc.sync.dma_start(out=fcgb_sb[:, 0, :], in_=d_fcgb[0:128, :])
        nc.sync.dma_start(out=fcgb_sb[:68, 1, :], in_=d_fcgb[128:196, :])

        # --- conv -> mapped shard -> DRAM (rank layout (s, n_local, m)).
        # Tiles are (n-pair x 49-s-chunk) = 98 rows, kc-outer inside each wave
        # so the matmuls start as soon as img chunk 0 lands; each s-chunk's
        # AllGather is issued the moment its 4 tiles are scattered.
        imv = img_sb.rearrange("p k (n s) -> p k n s", s=E)
        for sc in range(2):
            gtl = [psum.tile([98, 512], F32, name=f"cvg{sc}_{i}", tag="g", bufs=4)
                   for i in range(4)]
            mvt = [psum.tile([98, 2, 512], F32, name=f"cvm{sc}_{i}", tag="mv",
                             bufs=2) for i in range(2)]
            tiles = [(n_, gtl[n_] if n_ < 4 else mvt[(n_ - 4) // 2][:, n_ % 2, :])
                     for n_ in range(NL)]
            for kc in range(16):
                for n_, out in tiles:
                    mm(out=out, lhsT=imv[:, kc, n_, SCH * sc:SCH * (sc + 1)],
                       rhs=convw_sb[:, kc, :], start=(kc == 0), stop=False)
            for n_, out in tiles:
                mm(out=out, lhsT=ones1[:, :98], rhs=convw_sb[0:1, 16, :],
                   start=False, stop=True)
                ccast = initp.tile([98, M], F8, name="ccast", bufs=3)
                nc.vector.tensor_copy(out=ccast, in_=out)
                dst = bass.AP(tensor=d_agm_in.tensor,
                              offset=((SCH * sc) * NL + n_) * M,
                              ap=[[NL * M, SCH], [1, M]])
                nc.sync.dma_start(out=dst, in_=ccast)
            nc.gpsimd.collective_compute(
                "AllGather", OP.bypass, replica_groups=RG,
                ins=[d_agm_in[SCH * sc * NL * M: (SCH * sc + SCH) * NL * M]],
                outs=[d_agm_outs[sc][:]])

        # --- g = mean_s(img) @ fcg_w.T + fcg_b  (via P = fcg_w @ img_t, reduce s)
        for mt, (m0, msz) in enumerate([(0, 128), (128, 68)]):
            p01 = mvtile([128, 2, 512], F32, "p01")
            p23 = mvtile([128, 2, 512], F32, "p23")
            tgt = [(p01, 0), (p01, 1), (p23, 0), (p23, 1)]
            for kc in range(16):
                for nt in range(4):
                    pt, sl = tgt[nt]
                    mm(out=pt[:msz, sl, :GNT], lhsT=fcgw_sb[:, kc, m0:m0 + msz],
                       rhs=img_sb[:, kc, GNT * nt:GNT * (nt + 1)],
                       start=(kc == 0), stop=(kc == 15))
            gpre = initp.tile([128, 8], F32, name="gpre", bufs=2)
            for half, pt in enumerate((p01, p23)):
                src = pt[:msz, :, :GNT].rearrange("p a (b s) -> p a b s", s=E)
                nc.vector.tensor_reduce(out=gpre[:msz, 4 * half:4 * half + 4],
                                        in_=src, axis=mybir.AxisListType.X,
                                        op=OP.add)
            g16 = initp.tile([128, 8], F16, name="g16", bufs=2)
            nc.scalar.activation(out=g16[:msz, :], in_=gpre[:msz, :], func=AF.Identity,
                                 bias=fcgb_sb[:msz, mt, :], scale=1.0 / E)
            dst = bass.AP(tensor=d_agg_in.tensor, offset=m0 * NL,
                          ap=[[NL, msz], [1, NL]])
            nc.sync.dma_start(out=dst, in_=g16[:msz, :])

        nc.gpsimd.collective_compute("AllGather", OP.bypass, replica_groups=RG,
                                     ins=[d_agg_in[:]], outs=[d_agg_out[:]])

        # --- embedding gather + transpose
        seq_sb = initp.tile([128, 12], I32, name="seq_sb")
        nc.sync.dma_start(out=seq_sb,
                          in_=bass.AP(tensor=d_seq.tensor, offset=0,
                                      ap=[[1, 128], [128, 12]]))
        e_all = initp.tile([128, 12, E], F32, name="e_all")
        for b in range(12):
            nc.gpsimd.indirect_dma_start(
                out=e_all[:, b, :], out_offset=None, in_=d_emb[:],
                in_offset=bass.IndirectOffsetOnAxis(ap=seq_sb[:, b:b + 1], axis=0))
        for b in range(12):
            etp = mvtile([128, 2, 128], F32, "etp")
            nc.tensor.transpose(out=etp[:, 0, :], in_=e_all[:, b, 0:128], identity=idn32)
            nc.tensor.transpose(out=etp[:68, 1, :], in_=e_all[:, b, 128:196],
                                identity=idn32)
            nc.vector.tensor_copy(out=e_allT[:, 0, 128 * b:128 * (b + 1)],
                                  in_=etp[:, 0, :])
            nc.vector.tensor_copy(out=e_allT[:68, 1, 128 * b:128 * (b + 1)],
                                  in_=etp[:68, 1, :])
        nc.gpsimd.dma_start(out=e_allT[68:69, 1, :], in_=ones1[:, :T * N])

        initp.release()

        mappool = tc.alloc_tile_pool(name="mappool", bufs=1)
        mapped = mappool.tile([128, 2, N, M], F8, name="mapped")
        # chunk 0: s 0..97 -> cchunk0 rows 0..97; chunk 1: s 98..195 ->
        # cchunk0 rows 98..127 (30) + cchunk1 rows 0..67 (68)
        for k, pieces in enumerate([[(0, 0, 0, 98)], [(0, 98, 0, 30), (1, 0, 30, 68)]]):
            for r in range(NCORES):
                for cchunk, row0, off, cnt in pieces:
                    src = bass.AP(tensor=d_agm_outs[k].tensor,
                                  offset=(r * SCH + off) * NL * M,
                                  ap=[[NL * M, cnt], [M, NL], [1, M]])
                    nc.sync.dma_start(
                        out=mapped[row0:row0 + cnt, cchunk, NL * r:NL * (r + 1), :],
                        in_=src)
        for half, (e0, ecnt) in enumerate([(0, 128), (128, 68)]):
            src = bass.AP(tensor=d_agg_out.tensor, offset=e0 * NL,
                          ap=[[NL, ecnt], [E * NL, NCORES], [1, NL]])
            nc.sync.dma_start(out=g_allT[:ecnt, half, :], in_=src)

        # ---------- step machinery ----------
        def ctx_mvs(lhsT_tile, blkA, blkB, Asz=128, Bsz=68):
            """ctx_raw[n,:] = mapped[n] @ col_n(lhsT); returns dense ctx_raw.

            Row n = 8p + 2j + s runs on col-group j, psum-tile p, slot s, so
            the sparse psum rows (partitions 0/32/64/96) re-pack densely with
            one affine SBUF->SBUF DMA per tile (DMA cannot read PSUM; DVE/ACT
            evacuate partition-preserving first).
            """
            ctx_raw = work.tile([N, M], F16, name="ctx_raw", tag="ctx_raw")
            for p in range(8):
                mv = mvtile([128, 2, 512], F32, "mv")
                for s in range(2):
                    for j in range(4):
                        n_ = 8 * p + 2 * j + s
                        for c, (blk, cnt) in enumerate(((blkA, Asz), (blkB, Bsz))):
                            mm(out=mv[32 * j:32 * j + 32, s, :],
                               lhsT=lhsT_tile[:cnt, blk, n_:n_ + 1].to_broadcast(
                                   [cnt, 32]),
                               rhs=mapped[:cnt, c, n_, :],
                               start=(c == 0), stop=(c == 1),
                               tile_position=(0, 32 * j))
                sp = work.tile([128, 2, 512], F16, name="sp", tag="sp", bufs=2)
                if p % 2 == 0:
                    nc.vector.tensor_copy(out=sp, in_=mv)
                else:
                    nc.scalar.copy(out=sp, in_=mv)
                nc.sync.dma_start(out=ctx_raw[8 * p:8 * p + 8, :],
                                  in_=sp[0:128:32, :, :])
            return ctx_raw

        def ctx_norm(ctx_raw, parity):
            """l2-normalize ctx_raw and transpose into ctxT[parity].

            rsqrt = int-magic seed + 1 Newton step (rel err ~2e-3, plenty for a
            normalized direction vector); the per-sample scale rides into the
            PE transposes as a diagonal 'identity' so no 512-wide scale op.
            """
            sq = work.tile([N, M], F16, name="sq", tag="sq")
            q = tiny.tile([N, 1], F32, name="q", tag="q")
            nc.vector.scalar_tensor_tensor(out=sq, in0=ctx_raw, scalar=0.0,
                                           in1=ctx_raw, op0=OP.add, op1=OP.mult,
                                           accum_out=q)
            yi = tiny.tile([N, 1], I32, name="yi", tag="yi")
            nc.vector.tensor_scalar(out=yi, in0=q.bitcast(I32), scalar1=1,
                                    scalar2=None, op0=OP.logical_shift_right)
            nc.vector.tensor_scalar(out=yi, in0=yi, scalar1=0x5f375a86,
                                    scalar2=-1, op0=OP.subtract, op1=OP.mult)
            y = yi.bitcast(F32)
            t1 = tiny.tile([N, 1], F32, name="t1", tag="t1")
            nc.vector.tensor_tensor(out=t1, in0=y, in1=y, op=OP.mult)
            nc.vector.tensor_tensor(out=t1, in0=t1, in1=q, op=OP.mult)
            nc.vector.tensor_scalar(out=t1, in0=t1, scalar1=-0.5, scalar2=1.5,
                                    op0=OP.mult, op1=OP.add)
            nc.vector.tensor_tensor(out=t1, in0=y, in1=t1, op=OP.mult)
            ctx16 = work.tile([N, M], F16, name="ctx16", tag="ctx16")
            nc.vector.tensor_scalar(out=ctx16, in0=ctx_raw, scalar1=t1,
                                    scalar2=None, op0=OP.mult)
            tpc = mvtile([128, 4, N], F16, "tpc")
            for b in range(4):
                nc.tensor.transpose(out=tpc[:, b, :], in_=ctx16[:, 128 * b:128 * (b + 1)],
                                    identity=idn16[0:N, 0:N])
                nc.vector.tensor_copy(out=ctxT[:, parity, b, :], in_=tpc[:, b, :])

        def l0_eh_mms(t):
            """Open gates0 psum tiles for step t; accumulate emb+h parts."""
            t64 = t * N
            tiles = []
            for d in range(2):
                for sub in range(2):
                    ps = gtile(f"g0d{d}s{sub}")
                    tiles.append(ps)
                    col = d * 784 + sub * GNT
                    out = ps[:, :GNT]
                    seqm = [(e_allT[:, 0, t64:t64 + N], w0e[:, 0, col:col + GNT]),
                            (e_allT[:69, 1, t64:t64 + N], w0e[:69, 1, col:col + GNT])]
                    for cb, (blk, cnt) in enumerate(((2 * d, 128), (2 * d + 1, 68))):
                        seqm.append((h0T[:cnt, blk, :], w0h[:cnt, cb, col:col + GNT]))
                    for i, (lh, rh) in enumerate(seqm):
                        mm(out=out, lhsT=lh, rhs=rh, start=(i == 0), stop=False)
            return tiles

        def l0_ctx_mms(tiles, parity):
            for d in range(2):
                for sub in range(2):
                    ps = tiles[2 * d + sub]
                    col = d * 784 + sub * GNT
                    out = ps[:, :GNT]
                    for k in range(4):
                        mm(out=out, lhsT=ctxT[:, parity, k, :],
                           rhs=w0c[:, k, col:col + GNT],
                           start=False, stop=(k == 3))

        def l1_mms(t):
            tiles = []
            for d in range(2):
                for sub in range(2):
                    ps = gtile(f"g1d{d}s{sub}")
                    tiles.append(ps)
                    col = d * 784 + sub * GNT
                    out = ps[:, :GNT]
                    seqm = []
                    for b, (r0, sz) in enumerate(HBLK):
                        szx = sz + 1 if b == 3 else sz  # include ones row
                        seqm.append((h0T[:szx, b, :], w1x[:szx, b, col:col + GNT]))
                    for cb, (blk, cnt) in enumerate(((2 * d, 128), (2 * d + 1, 68))):
                        seqm.append((h1T[:cnt, blk, :], w1h[:cnt, cb, col:col + GNT]))
                    last = len(seqm) - 1
                    for i, (lh, rh) in enumerate(seqm):
                        mm(out=out, lhsT=lh, rhs=rh, start=(i == 0), stop=(i == last))
            return tiles

        def gates_tanh(tiles, layer):
            Tg = work.tile([N, 4, GNT], F16, name=f"T{layer}", tag=f"T{layer}")
            for d in range(2):
                for sub in range(2):
                    nc.scalar.activation(out=Tg[:, 2 * d + sub, :],
                                         in_=tiles[2 * d + sub][:, :GNT],
                                         func=AF.Tanh)
            return Tg

        def cell_dve(Tg, layer):
            """fp16 cell math; returns hh [N, 2E] (h~ row layout)."""
            Ct = Ct0 if layer == 0 else Ct1
            T_i = Tg[:, 0::2, 0:E]
            T_f = Tg[:, 0::2, E:2 * E]
            T_o = Tg[:, 1::2, 0:E]
            T_g = Tg[:, 1::2, E:2 * E]
            u = work.tile([N, 2, E], F16, name="u", tag="u")
            sf = work.tile([N, 2, E], F16, name="sf", tag="sf")
            nc.vector.scalar_tensor_tensor(out=u, in0=T_i, scalar=1.0, in1=T_g,
                                           op0=OP.add, op1=OP.mult)
            nc.vector.tensor_scalar(out=sf, in0=T_f, scalar1=0.5, scalar2=0.5,
                                    op0=OP.mult, op1=OP.add)
            nc.vector.tensor_tensor(out=sf, in0=sf, in1=Ct, op=OP.mult)
            nc.vector.tensor_tensor(out=Ct, in0=u, in1=sf, op=OP.add)
            Tc = work.tile([N, 2, E], F16, name=f"Tc{layer}", tag="Tc")
            nc.scalar.activation(out=Tc, in_=Ct, func=AF.Tanh, scale=0.5)
            hh = work.tile([N, 2 * E], F16, name=f"h{layer}_", tag=f"h{layer}_")
            hhv = hh.rearrange("p (a b) -> p a b", a=2)
            nc.vector.scalar_tensor_tensor(out=hhv, in0=T_o, scalar=1.0, in1=Tc,
                                           op0=OP.add, op1=OP.mult)
            return hh

        def h_transpose(hh, hT, layer):
            tph = mvtile([128, 4, N], F16, f"tph{layer}")
            for b, (c0, w) in enumerate(HBLK):
                nc.tensor.transpose(out=tph[:w, b, :], in_=hh[:, c0:c0 + w],
                                    identity=idn16[0:N, 0:N])
                nc.vector.tensor_copy(out=hT[:w, b, :], in_=tph[:w, b, :])

        def lin_mms(parity):
            lps = mvtile([N, 512], F32, "lps")
            seqm = []
            for b, (r0, sz) in enumerate(HBLK):
                szx = sz + 1 if b == 3 else sz
                seqm.append((h1T[:szx, b, :], lin_sb[:szx, b, :]))
            for k in range(4):
                seqm.append((ctxT[:, parity, k, :], lin_sb[:, 4 + k, :]))
            for i, (lh, rh) in enumerate(seqm):
                mm(out=lps, lhsT=lh, rhs=rh, start=(i == 0),
                   stop=(i == len(seqm) - 1))
            return lps

        def lin_leaky(lps):
            # leaky_relu(x) = 0.01x + relu(0.99x), split ACT (relu) + DVE (mix)
            r99 = work.tile([N, M], F16, name="r99", tag="r99")
            nc.scalar.activation(out=r99, in_=lps, func=AF.Relu, scale=0.99)
            a16 = work.tile([N, M], F16, name="a16", tag="a16")
            nc.vector.scalar_tensor_tensor(out=a16, in0=lps, scalar=0.01,
                                           in1=r99, op0=OP.mult, op1=OP.add)
            return a16

        def a_transpose(a16):
            tpa = mvtile([128, 4, N], F16, "tpa")
            for b in range(4):
                nc.tensor.transpose(out=tpa[:, b, :], in_=a16[:, 128 * b:128 * (b + 1)],
                                    identity=idn16[0:N, 0:N])
                nc.vector.tensor_copy(out=aT[:, b, :], in_=tpa[:, b, :])

        def vocab_A(t):
            """nt0 matmuls (5), exp+accum, raw-logit stash of cols 0..511."""
            vpsA = mvtile([N, 512], F32, "vpsA")
            v0, w = VOC_NT[0]
            for k in range(5):
                cnt = 128 if k < 4 else 1
                mm(out=vpsA, lhsT=aT[:cnt, k, :],
                   rhs=wp_sb[:cnt, k, v0:v0 + w], start=(k == 0), stop=(k == 4))
            xraw = work.tile([N, LRAW_W], F16, name="xraw", tag="xraw", bufs=2)
            xv = xraw.rearrange("p (a b) -> p a b", a=3)
            dump = work.tile([N, LRAW_W], F16, name="dump", tag="dump", bufs=2)
            dv = dump.rearrange("p (a b) -> p a b", a=3)
            s1 = tiny.tile([N, 1], F32, name="s1", tag="s1")
            nc.scalar.activation(out=dv[:, 0, :], in_=vpsA, func=AF.Exp,
                                 accum_out=s1)
            nc.vector.tensor_copy(out=xv[:, 0, :], in_=vpsA)
            return xraw, dump, s1

        def vocab_B(t, xraw, dump, s1):
            """nt1+nt2 matmuls (10), exp+accum, stash, s-total, DRAM spill."""
            vpsB = mvtile([N, 2, 512], F32, "vpsB")
            for sl, nt in enumerate((1, 2)):
                v0, w = VOC_NT[nt]
                for k in range(5):
                    cnt = 128 if k < 4 else 1
                    mm(out=vpsB[:, sl, :w], lhsT=aT[:cnt, k, :],
                       rhs=wp_sb[:cnt, k, v0:v0 + w], start=(k == 0), stop=(k == 4))
            xv = xraw.rearrange("p (a b) -> p a b", a=3)
            dv = dump.rearrange("p (a b) -> p a b", a=3)
            s2 = tiny.tile([N, 1], F32, name="s2", tag="s2")
            nc.scalar.activation(out=dv[:, 1, :], in_=vpsB[:, 0, :], func=AF.Exp,
                                 accum_out=s2)
            s3 = tiny.tile([N, 1], F32, name="s3", tag="s3")
            nc.scalar.activation(out=dv[:, 2, :476], in_=vpsB[:, 1, :476],
                                 func=AF.Exp, accum_out=s3)
            nc.vector.tensor_copy(out=xv[:, 1, :], in_=vpsB[:, 0, :])
            nc.vector.tensor_copy(out=xv[:, 2, :476], in_=vpsB[:, 1, :476])
            nc.vector.tensor_tensor(out=s2, in0=s2, in1=s3, op=OP.add)
            nc.vector.tensor_tensor(out=sAll[:, t:t + 1], in0=s1, in1=s2, op=OP.add)
            nc.sync.dma_start(out=d_lraw[t][:, :1500], in_=xraw[:, :1500])

        # ---------- initial context (writes parity 1) ----------
        craw = ctx_mvs(g_allT, 0, 1)
        g0_first = l0_eh_mms(0)          # fills the initial ctx-norm tail
        ctx_norm(craw, 1)

        # ---------- software-pipelined steps ----------
        # Loop iteration t interleaves step t's critical path (gates0 -> cell0
        # -> gates1 -> cell1 -> ctx) with step t-1's head (aT/vocab/exp) in the
        # cell gaps and step t+1's gates0-partial + step t's lin in the
        # ctx-norm tail, so the PE stays fed through every serial chain.
        g0_tiles = g0_first
        a16 = None
        for t in range(n_steps):
            l0_ctx_mms(g0_tiles, (t - 1) % 2)            # finish gates0(t)
            Tg0 = gates_tanh(g0_tiles, 0)
            if t > 0:
                a_transpose(a16)                         # aT(t-1)
                vA = vocab_A(t - 1)                      # fills cell0 gap
            hh0 = cell_dve(Tg0, 0)
            h_transpose(hh0, h0T, 0)
            g1_tiles = l1_mms(t)
            Tg1 = gates_tanh(g1_tiles, 1)
            if t > 0:
                vocab_B(t - 1, *vA)                      # fills cell1 gap
            hh1 = cell_dve(Tg1, 1)
            h_transpose(hh1, h1T, 1)
            craw = ctx_mvs(h1T, 2, 3)
            if t + 1 < n_steps:
                g0_tiles = l0_eh_mms(t + 1)              # fills ctx-norm tail
            lps = lin_mms((t - 1) % 2)                   # lin(t), tail fill too
            ctx_norm(craw, t % 2)
            a16 = lin_leaky(lps)

        # trailing head for the last step
        a_transpose(a16)
        vA = vocab_A(n_steps - 1)
        vocab_B(n_steps - 1, *vA)

        mappool.release()

        # ---------- finale: AllReduce s, ln, subtract ----------
        nc.sync.dma_start(out=bass.AP(tensor=d_s_in.tensor, offset=0,
                                      ap=[[T, N], [1, T]]), in_=sAll)
        nc.gpsimd.collective_compute("AllReduce", OP.add, replica_groups=RG,
                                     ins=[d_s_in[:]], outs=[d_s_out[:]])
        finp = tc.alloc_tile_pool(name="finp", bufs=3)
        sg = state.tile([N, T], F32, name="sg")
        nc.sync.dma_start(out=sg, in_=bass.AP(tensor=d_s_out.tensor, offset=0,
                                              ap=[[T, N], [1, T]]))
        lns = state.tile([N, T], F32, name="lns")
        nc.scalar.activation(out=lns, in_=sg, func=AF.Ln)
        nlns = state.tile([N, T], F32, name="nlns")
        nc.vector.tensor_scalar(out=nlns, in0=lns, scalar1=-1.0, scalar2=None,
                                op0=OP.mult)
        for t in range(T):
            xst = finp.tile([N, LRAW_W], F16, name="xst", tag="xst")
            nc.sync.dma_start(out=xst[:, :1500], in_=d_lraw[t][:, :1500])
            ot = finp.tile([N, VS], F32, name="ot", tag="ot")
            if t % 2 == 0:
                nc.vector.tensor_scalar(out=ot, in0=xst[:, 0:VS],
                                        scalar1=lns[:, t:t + 1],
                                        scalar2=None, op0=OP.subtract)
            else:
                nc.scalar.activation(out=ot, in_=xst[:, 0:VS], func=AF.Identity,
                                     bias=nlns[:, t:t + 1])
            nc.sync.dma_start(out=d_out[t], in_=ot)
        finp.release()
        for p in (psum, tiny, work, state, wpool):
            p.release()
    return nc


_CACHED = {}


def _build_nc(n_steps=T):
    key = ("nc", n_steps)
    if key not in _CACHED:
        nc = bacc.Bacc("TRN2", target_bir_lowering=False, debug=False,
                       num_devices=NCORES)
        build(nc, n_steps)
        nc.compile()
        _CACHED[key] = nc
    return _CACHED[key]


def run(inputs, trace=False):
    nc = _build_nc()
    in_maps = prepare_inputs(inputs)
    res = run_bass_kernel_spmd(nc, in_maps, list(range(NCORES)), trace=trace)
    out = np.concatenate([res.results[r]["out_logits"] for r in range(NCORES)],
                         axis=2)
    return out.astype(np.float32), res


def kernel(**inputs):
    out, _ = run(inputs, trace=False)
    return out


# revision 30
# speedup vs baseline: 1.2648x; 1.0535x over previous
"""Trainium2 Bass kernel for nn_Caption (bidirectional-LSTM image captioner).

Distribution over 8 NeuronCores (zero per-step collectives):
  - Recurrent computation (both LSTM layers, lin, context attention) is
    REPLICATED on all cores with the full batch of 64: per-step gate matmuls
    are PE-streaming-bound (cost independent of batch <= 128), so replication
    is free and avoids per-step collectives (AllGather floor ~5us x 24 steps).
  - Vocab projection (12000) is sharded 8-way (1500 cols/core).
  - The 1x1 conv ("mapped") is sharded by batch (8 rows/core) and AllGathered
    once (fp16) at the start; every core holds the full mapped for the
    per-step context matvecs.
  - log_softmax: logits are tiny so no max-subtraction is needed; each core
    accumulates per-(t,n) sum of exp over its vocab slice; ONE AllReduce of
    (64,24) sums at the end; final pass writes x - ln(s_global).

Per-step schedule is software-pipelined so the PE never idles during the
cell-math / l2norm chains: lin+vocab of step t-1 are issued into step t's
gate phases, and gates0 of step t+1 is split into an (emb,h) partial
(issued during step t's ctx-norm tail) plus a ctx part once ctxT lands.

Layout: all matmuls are activation-stationary (lhsT = activations^T), so
activations are transposed each step via PE transposes.  Biases ride as
extra contraction rows against constant-1 rows in the transposed
activations.  sigma(x)=0.5*tanh(x/2)+0.5 with the 0.5 pre-scaled into the
i/f/o weight columns so one plain tanh covers all gates.  Cell state is kept
scaled (Ct=2c, h~=2h) with 0.5 folded into downstream weights; the
l2-normalized ctx is invariant to the h~ scaling.
"""

import sys
import numpy as np
import ml_dtypes

for _p in ("/opt/trn_rl_repo",):
    if _p not in sys.path:
        sys.path.insert(0, _p)

import concourse.bass as bass
import concourse.tile as tile
from concourse import bacc
from concourse import mybir
from concourse.masks import make_identity
from concourse.bass_utils import run_bass_kernel_spmd

F16 = mybir.dt.float16
F8 = mybir.dt.float8e4
F32 = mybir.dt.float32
I32 = mybir.dt.int32
AF = mybir.ActivationFunctionType
OP = mybir.AluOpType

N = 64          # batch
T = 24          # steps
E = 196         # embedding/hidden size
M = 512         # context dim
C = 2048        # image channels
V = 12000       # vocab
NCORES = 8
VS = V // NCORES          # vocab slice per core
NL = N // NCORES          # batch rows per core (conv shard)
NS = NL * E               # conv rows per core (1568)
G2 = 2 * 4 * E            # gate cols, both dirs (1568)
RG = [list(range(NCORES))]
GNT = 392                 # gates N-tile
VOC_NT = [(0, 512), (512, 512), (1024, 476)]
LRAW_W = 1536             # padded row width of raw-logit staging

# h^T tiles are blocked {128, 68, 128, 68(+ones)} so fwd/bwd chunks align.
HBLK = [(0, 128), (128, 68), (196, 128), (324, 68)]


def _f16(x):
    return np.ascontiguousarray(x, dtype=np.float16)


def _f32(x):
    return np.ascontiguousarray(x, dtype=np.float32)


def prepare_inputs(inputs):
    img = _f32(np.asarray(inputs["input_image_feat"])).reshape(N, E, C)
    seq = np.ascontiguousarray(np.asarray(inputs["sequences"]).astype(np.int32))
    conv_w = _f32(inputs["conv_w"]); conv_b = _f32(inputs["conv_b"])
    fcg_w = _f32(inputs["fcg_w"]); fcg_b = _f32(inputs["fcg_b"])
    emb = _f32(inputs["emb"])
    w_ih0 = _f32(inputs["w_ih0"]); w_hh0 = _f32(inputs["w_hh0"]); b0 = _f32(inputs["b0"])
    w_ih1 = _f32(inputs["w_ih1"]); w_hh1 = _f32(inputs["w_hh1"]); b1 = _f32(inputs["b1"])
    lin_w = _f32(inputs["lin_w"]); lin_b = _f32(inputs["lin_b"])
    wp_w = _f32(inputs["wp_w"]); wp_b = _f32(inputs["wp_b"])

    # gate reorder [i f g o] -> [i f o g]; pre-scale i/f/o columns by 0.5
    perm = np.r_[0:E, E:2 * E, 3 * E:4 * E, 2 * E:3 * E]
    gsc = np.ones(4 * E, np.float32)
    gsc[: 3 * E] = 0.5

    def gmat(w):            # (784, in) -> (in, 784) permuted + scaled
        return w.T[:, perm] * gsc

    def gvec(b):
        return b[perm] * gsc

    W0 = np.concatenate([gmat(w_ih0[0]), gmat(w_ih0[1])], axis=1)        # (708,1568)
    b0r = np.concatenate([gvec(b0[0]), gvec(b0[1])])
    W0e = _f16(np.concatenate([W0[:E], b0r[None]], axis=0))              # (197,1568)
    W0c = _f16(W0[E:E + M])                                              # (512,1568)
    W0h = _f16(0.5 * np.concatenate([gmat(w_hh0[0]), gmat(w_hh0[1])], 1))  # (196,1568)
    W1 = 0.5 * np.concatenate([gmat(w_ih1[0]), gmat(w_ih1[1])], axis=1)  # (392,1568)
    b1r = np.concatenate([gvec(b1[0]), gvec(b1[1])])
    W1x = _f16(np.concatenate([W1, b1r[None]], axis=0))                  # (393,1568)
    W1h = _f16(0.5 * np.concatenate([gmat(w_hh1[0]), gmat(w_hh1[1])], 1))  # (196,1568)
    lin_aug = _f16(np.concatenate(                                       # (905,512)
        [0.5 * lin_w.T[:2 * E], lin_b[None], lin_w.T[2 * E:]], axis=0))
    conv_wT_aug = _f16(np.concatenate([conv_w.T, conv_b[None]], axis=0))  # (2049,512)

    base = dict(
        W0e=W0e, W0c=W0c, W0h=W0h, W1x=W1x, W1h=W1h, lin_aug=lin_aug,
        conv_wT_aug=conv_wT_aug, fcg_wT=_f16(fcg_w.T),
        fcg_b=_f32(fcg_b.reshape(E, 1)), emb=emb,
        seq_idx=np.ascontiguousarray(seq.reshape(T * N, 1)),
    )
    in_maps = []
    for r in range(NCORES):
        m = dict(base)
        m["img_t"] = np.ascontiguousarray(
            img[NL * r: NL * (r + 1)].reshape(NS, C).T.astype(ml_dtypes.float8_e4m3))
        m["wp_aug"] = _f16(np.concatenate(
            [wp_w[VS * r: VS * (r + 1)].T, wp_b[None, VS * r: VS * (r + 1)]], axis=0))
        in_maps.append(m)
    return in_maps


def build(nc, n_steps=T):
    mm = nc.tensor.matmul
    d_img = nc.dram_tensor("img_t", [C, NS], F8, kind="ExternalInput").ap()
    d_convw = nc.dram_tensor("conv_wT_aug", [C + 1, M], F16, kind="ExternalInput").ap()
    d_fcgw = nc.dram_tensor("fcg_wT", [C, E], F16, kind="ExternalInput").ap()
    d_fcgb = nc.dram_tensor("fcg_b", [E, 1], F32, kind="ExternalInput").ap()
    d_emb = nc.dram_tensor("emb", [V, E], F32, kind="ExternalInput").ap()
    d_seq = nc.dram_tensor("seq_idx", [T * N, 1], I32, kind="ExternalInput").ap()
    d_w0e = nc.dram_tensor("W0e", [E + 1, G2], F16, kind="ExternalInput").ap()
    d_w0c = nc.dram_tensor("W0c", [M, G2], F16, kind="ExternalInput").ap()
    d_w0h = nc.dram_tensor("W0h", [E, G2], F16, kind="ExternalInput").ap()
    d_w1x = nc.dram_tensor("W1x", [2 * E + 1, G2], F16, kind="ExternalInput").ap()
    d_w1h = nc.dram_tensor("W1h", [E, G2], F16, kind="ExternalInput").ap()
    d_lin = nc.dram_tensor("lin_aug", [2 * E + 1 + M, M], F16, kind="ExternalInput").ap()
    d_wp = nc.dram_tensor("wp_aug", [M + 1, VS], F16, kind="ExternalInput").ap()
    d_out = nc.dram_tensor("out_logits", [T, N, VS], F32, kind="ExternalOutput").ap()

    d_lraw = nc.dram_tensor("logits_raw", [T, N, LRAW_W], F16).ap()
    d_agm_in = nc.dram_tensor("agm_in", [E * NL * M], F8).ap()
    # mapped AllGather is chunked by s-range (98 rows each) so transfers
    # overlap the conv; each chunk's output is rank-major on its own buffer.
    SCH = 98
    d_agm_outs = [nc.dram_tensor(f"agm_out{k}", [NCORES * SCH * NL * M], F8,
                                 addr_space="Shared").ap() for k in range(2)]
    d_agg_in = nc.dram_tensor("agg_in", [E * NL], F16).ap()
    d_agg_out = nc.dram_tensor("agg_out", [NCORES * E * NL], F16,
                               addr_space="Shared").ap()
    d_s_in = nc.dram_tensor("s_in", [N * T], F32).ap()
    d_s_out = nc.dram_tensor("s_out", [N * T], F32, addr_space="Shared").ap()

    with tile.TileContext(nc) as tc:
        wpool = tc.alloc_tile_pool(name="wpool", bufs=1)
        state = tc.alloc_tile_pool(name="state", bufs=1)
        work = tc.alloc_tile_pool(name="work", bufs=1)
        tiny = tc.alloc_tile_pool(name="tiny", bufs=1)
        psum = tc.alloc_tile_pool(name="psum", bufs=1, space="PSUM")
        initp = tc.alloc_tile_pool(name="initp", bufs=1)

        # psum rings: "g" = 4 x [64,512] f32 (2KB) gate/lin tiles;
        #             "mv" = 2 x 4KB shared by ctx matvec / vocab / transposes.
        def gtile(name):
            return psum.tile([N, 512], F32, name=name, tag="g", bufs=4)

        def mvtile(shape, dt, name):
            return psum.tile(shape, dt, name=name, tag="mv", bufs=2)

        # ---------- persistent weights ----------
        def load_w(name, dram, blocks, width):
            t = wpool.tile([128, len(blocks), width], F16, name=name)
            for b, (r0, sz) in enumerate(blocks):
                nc.sync.dma_start(out=t[:sz, b, :], in_=dram[r0:r0 + sz, :])
            return t

        B128 = lambda rows: [(i, min(128, rows - i)) for i in range(0, rows, 128)]

        idn16 = wpool.tile([128, 128], F16, name="idn16")
        make_identity(nc, idn16)
        idn32 = wpool.tile([128, 128], F32, name="idn32")
        make_identity(nc, idn32)
        ones1 = wpool.tile([1, T * N], F16, name="ones1")
        nc.vector.memset(ones1, 1.0)

        e_allT = wpool.tile([128, 2, T * N], F16, name="e_allT")
        g_allT = wpool.tile([128, 2, N], F16, name="g_allT")

        # ---------- recurrent state ----------
        h0T = state.tile([128, 4, N], F16, name="h0T")
        h1T = state.tile([128, 4, N], F16, name="h1T")
        ctxT = state.tile([128, 2, 4, N], F16, name="ctxT")  # parity-double-buffered
        aT = state.tile([128, 5, N], F16, name="aT")
        Ct0 = state.tile([N, 2, E], F16, name="Ct0")
        Ct1 = state.tile([N, 2, E], F16, name="Ct1")
        sAll = state.tile([N, T], F32, name="sAll")
        for t_ in (ctxT, aT, Ct0, Ct1):
            nc.vector.memset(t_, 0.0)
        for t_ in (h0T, h1T):
            nc.vector.memset(t_[:, 0:3, :], 0.0)
            nc.vector.memset(t_[0:68, 3, :], 0.0)
        nc.gpsimd.dma_start(out=h0T[68:69, 3, :], in_=ones1[:, :N])
        nc.gpsimd.dma_start(out=h1T[68:69, 3, :], in_=ones1[:, :N])
        nc.vector.memset(aT[0:1, 4, :], 1.0)

        # ================= INIT =================
        img_sb = initp.tile([128, 16, NS], F8, name="img_sb")
        for kc in range(16):
            nc.sync.dma_start(out=img_sb[:, kc, :],
                              in_=d_img[128 * kc:128 * (kc + 1), :])
        convw_sb = initp.tile([128, 17, M], F16, name="convw_sb")
        for b, (r0, sz) in enumerate(B128(C) + [(C, 1)]):
            nc.sync.dma_start(out=convw_sb[:sz, b, :], in_=d_convw[r0:r0 + sz, :])
        fcgw_sb = initp.tile([128, 16, E], F16, name="fcgw_sb")
        for b, (r0, sz) in enumerate(B128(C)):
            nc.sync.dma_start(out=fcgw_sb[:sz, b, :], in_=d_fcgw[r0:r0 + sz, :])
        fcgb_sb = initp.tile([128, 2, 1], F32, name="fcgb_sb")
        nc.sync.dma_start(out=fcgb_sb[:, 0, :], in_=d_fcgb[0:128, :])
        nc.sync.dma_start(out=fcgb_sb[:68, 1, :], in_=d_fcgb[128:196, :])

        # --- conv -> mapped shard -> DRAM (rank layout (s, n_local, m)).
        # Tiles are (n-pair x 49-s-chunk) = 98 rows, kc-outer inside each wave
        # so the matmuls start as soon as img chunk 0 lands; each s-chunk's
        # AllGather is issued the moment its 4 tiles are scattered.
        imv = img_sb.rearrange("p k (n s) -> p k n s", s=E)
        for sc in range(2):
            gtl = [psum.tile([98, 512], F32, name=f"cvg{sc}_{i}", tag="g", bufs=4)
                   for i in range(4)]
            mvt = [psum.tile([98, 2, 512], F32, name=f"cvm{sc}_{i}", tag="mv",
                             bufs=2) for i in range(2)]
            tiles = [(n_, gtl[n_] if n_ < 4 else mvt[(n_ - 4) // 2][:, n_ % 2, :])
                     for n_ in range(NL)]
            for kc in range(16):
                for n_, out in tiles:
                    mm(out=out, lhsT=imv[:, kc, n_, SCH * sc:SCH * (sc + 1)],
                       rhs=convw_sb[:, kc, :], start=(kc == 0), stop=False)
            for n_, out in tiles:
                mm(out=out, lhsT=ones1[:, :98], rhs=convw_sb[0:1, 16, :],
                   start=False, stop=True)
                ccast = initp.tile([98, M], F8, name="ccast", bufs=3)
                nc.vector.tensor_copy(out=ccast, in_=out)
                dst = bass.AP(tensor=d_agm_in.tensor,
                              offset=((SCH * sc) * NL + n_) * M,
                              ap=[[NL * M, SCH], [1, M]])
                nc.sync.dma_start(out=dst, in_=ccast)
            nc.gpsimd.collective_compute(
                "AllGather", OP.bypass, replica_groups=RG,
                ins=[d_agm_in[SCH * sc * NL * M: (SCH * sc + SCH) * NL * M]],
                outs=[d_agm_outs[sc][:]])

        # --- g = mean_s(img) @ fcg_w.T + fcg_b  (via P = fcg_w @ img_t, reduce s)
        for mt, (m0, msz) in enumerate([(0, 128), (128, 68)]):
            p01 = mvtile([128, 2, 512], F32, "p01")
            p23 = mvtile([128, 2, 512], F32, "p23")
            tgt = [(p01, 0), (p01, 1), (p23, 0), (p23, 1)]
            for kc in range(16):
                for nt in range(4):
                    pt, sl = tgt[nt]
                    mm(out=pt[:msz, sl, :GNT], lhsT=fcgw_sb[:, kc, m0:m0 + msz],
                       rhs=img_sb[:, kc, GNT * nt:GNT * (nt + 1)],
                       start=(kc == 0), stop=(kc == 15))
            gpre = initp.tile([128, 8], F32, name="gpre", bufs=2)
            for half, pt in enumerate((p01, p23)):
                src = pt[:msz, :, :GNT].rearrange("p a (b s) -> p a b s", s=E)
                nc.vector.tensor_reduce(out=gpre[:msz, 4 * half:4 * half + 4],
                                        in_=src, axis=mybir.AxisListType.X,
                                        op=OP.add)
            g16 = initp.tile([128, 8], F16, name="g16", bufs=2)
            nc.scalar.activation(out=g16[:msz, :], in_=gpre[:msz, :], func=AF.Identity,
                                 bias=fcgb_sb[:msz, mt, :], scale=1.0 / E)
            dst = bass.AP(tensor=d_agg_in.tensor, offset=m0 * NL,
                          ap=[[NL, msz], [1, NL]])
            nc.sync.dma_start(out=dst, in_=g16[:msz, :])

        nc.gpsimd.collective_compute("AllGather", OP.bypass, replica_groups=RG,
                                     ins=[d_agg_in[:]], outs=[d_agg_out[:]])

        # --- persistent weights (issued late so they queue behind the
        # init-critical img/conv DMAs and overlap the AllGather wait)
        w0e = load_w("w0e", d_w0e, [(0, 128), (128, 69)], G2)
        w0c = load_w("w0c", d_w0c, B128(M), G2)
        w0h = load_w("w0h", d_w0h, [(0, 128), (128, 68)], G2)
        w1x = load_w("w1x", d_w1x, [(0, 128), (128, 68), (196, 128), (324, 69)], G2)
        w1h = load_w("w1h", d_w1h, [(0, 128), (128, 68)], G2)
        lin_sb = load_w("lin_sb", d_lin,
                        [(0, 128), (128, 68), (196, 128), (324, 69),
                         (393, 128), (521, 128), (649, 128), (777, 128)], M)
        wp_sb = load_w("wp_sb", d_wp, B128(M) + [(512, 1)], VS)

        # --- embedding gather + transpose
        seq_sb = initp.tile([128, 12], I32, name="seq_sb")
        nc.sync.dma_start(out=seq_sb,
                          in_=bass.AP(tensor=d_seq.tensor, offset=0,
                                      ap=[[1, 128], [128, 12]]))
        e_all = initp.tile([128, 12, E], F32, name="e_all")
        for b in range(12):
            nc.gpsimd.indirect_dma_start(
                out=e_all[:, b, :], out_offset=None, in_=d_emb[:],
                in_offset=bass.IndirectOffsetOnAxis(ap=seq_sb[:, b:b + 1], axis=0))
        for b in range(12):
            etp = mvtile([128, 2, 128], F32, "etp")
            nc.tensor.transpose(out=etp[:, 0, :], in_=e_all[:, b, 0:128], identity=idn32)
            nc.tensor.transpose(out=etp[:68, 1, :], in_=e_all[:, b, 128:196],
                                identity=idn32)
            nc.vector.tensor_copy(out=e_allT[:, 0, 128 * b:128 * (b + 1)],
                                  in_=etp[:, 0, :])
            nc.vector.tensor_copy(out=e_allT[:68, 1, 128 * b:128 * (b + 1)],
                                  in_=etp[:68, 1, :])
        nc.gpsimd.dma_start(out=e_allT[68:69, 1, :], in_=ones1[:, :T * N])

        initp.release()

        mappool = tc.alloc_tile_pool(name="mappool", bufs=1)
        mapped = mappool.tile([128, 2, N, M], F8, name="mapped")
        # chunk 0: s 0..97 -> cchunk0 rows 0..97; chunk 1: s 98..195 ->
        # cchunk0 rows 98..127 (30) + cchunk1 rows 0..67 (68)
        for k, pieces in enumerate([[(0, 0, 0, 98)], [(0, 98, 0, 30), (1, 0, 30, 68)]]):
            for r in range(NCORES):
                for cchunk, row0, off, cnt in pieces:
                    src = bass.AP(tensor=d_agm_outs[k].tensor,
                                  offset=(r * SCH + off) * NL * M,
                                  ap=[[NL * M, cnt], [M, NL], [1, M]])
                    nc.sync.dma_start(
                        out=mapped[row0:row0 + cnt, cchunk, NL * r:NL * (r + 1), :],
                        in_=src)
        for half, (e0, ecnt) in enumerate([(0, 128), (128, 68)]):
            src = bass.AP(tensor=d_agg_out.tensor, offset=e0 * NL,
                          ap=[[NL, ecnt], [E * NL, NCORES], [1, NL]])
            nc.sync.dma_start(out=g_allT[:ecnt, half, :], in_=src)

        # ---------- step machinery ----------
        def ctx_mvs(lhsT_tile, blkA, blkB, Asz=128, Bsz=68):
            """ctx_raw[n,:] = mapped[n] @ col_n(lhsT); returns (ctx_raw, q).

            Row n = 8p + 2j + s runs on col-group j, psum-tile p, slot s, so
            the sparse psum rows (partitions 0/32/64/96) re-pack densely with
            one affine SBUF->SBUF DMA per tile (DMA cannot read PSUM; DVE+ACT
            evacuate one slot each in parallel so the evac keeps pace with the
            matvecs).  The sum-of-squares accumulates in two slices so only
            the last 8 rows wait on the final repack DMA.
            """
            ctx_raw = work.tile([N, M], F16, name="ctx_raw", tag="ctx_raw")
            sq = work.tile([N, M], F16, name="sq", tag="sq")
            q = tiny.tile([N, 1], F32, name="q", tag="q")
            for p in range(8):
                mv = mvtile([128, 2, 512], F32, "mv")
                for s in range(2):
                    for j in range(4):
                        n_ = 8 * p + 2 * j + s
                        for c, (blk, cnt) in enumerate(((blkA, Asz), (blkB, Bsz))):
                            mm(out=mv[32 * j:32 * j + 32, s, :],
                               lhsT=lhsT_tile[:cnt, blk, n_:n_ + 1].to_broadcast(
                                   [cnt, 32]),
                               rhs=mapped[:cnt, c, n_, :],
                               start=(c == 0), stop=(c == 1),
                               tile_position=(0, 32 * j))
                sp = work.tile([128, 2, 512], F16, name="sp", tag="sp", bufs=2)
                nc.vector.tensor_copy(out=sp[:, 0, :], in_=mv[:, 0, :])
                nc.scalar.copy(out=sp[:, 1, :], in_=mv[:, 1, :])
                nc.sync.dma_start(out=ctx_raw[8 * p:8 * p + 8, :],
                                  in_=sp[0:128:32, :, :])
                if p == 3:
                    nc.vector.scalar_tensor_tensor(
                        out=sq[0:32], in0=ctx_raw[0:32], scalar=0.0,
                        in1=ctx_raw[0:32], op0=OP.add, op1=OP.mult,
                        accum_out=q[0:32])
            nc.vector.scalar_tensor_tensor(
                out=sq[32:64], in0=ctx_raw[32:64], scalar=0.0,
                in1=ctx_raw[32:64], op0=OP.add, op1=OP.mult, accum_out=q[32:64])
            return ctx_raw, q

        def ctx_norm(ctx_raw, q, parity):
            """l2-normalize ctx_raw and transpose into ctxT[parity].

            rsqrt = int-magic seed + 1 Newton step (rel err ~2e-3, plenty for
            a normalized direction vector).
            """
            yi = tiny.tile([N, 1], I32, name="yi", tag="yi")
            nc.vector.tensor_scalar(out=yi, in0=q.bitcast(I32), scalar1=1,
                                    scalar2=None, op0=OP.logical_shift_right)
            nc.vector.tensor_scalar(out=yi, in0=yi, scalar1=0x5f375a86,
                                    scalar2=-1, op0=OP.subtract, op1=OP.mult)
            y = yi.bitcast(F32)
            t1 = tiny.tile([N, 1], F32, name="t1", tag="t1")
            nc.vector.tensor_tensor(out=t1, in0=y, in1=y, op=OP.mult)
            nc.vector.tensor_tensor(out=t1, in0=t1, in1=q, op=OP.mult)
            nc.vector.tensor_scalar(out=t1, in0=t1, scalar1=-0.5, scalar2=1.5,
                                    op0=OP.mult, op1=OP.add)
            nc.vector.tensor_tensor(out=t1, in0=y, in1=t1, op=OP.mult)
            ctx16 = work.tile([N, M], F16, name="ctx16", tag="ctx16")
            nc.vector.tensor_scalar(out=ctx16, in0=ctx_raw, scalar1=t1,
                                    scalar2=None, op0=OP.mult)
            tpc = mvtile([128, 4, N], F16, "tpc")
            for b in range(4):
                nc.tensor.transpose(out=tpc[:, b, :], in_=ctx16[:, 128 * b:128 * (b + 1)],
                                    identity=idn16[0:N, 0:N])
                nc.vector.tensor_copy(out=ctxT[:, parity, b, :], in_=tpc[:, b, :])

        def l0_eh_mms(t, dirs=(0, 1), tiles=None):
            """Open gates0 psum tiles for step t; accumulate emb+h parts."""
            t64 = t * N
            if tiles is None:
                tiles = [None] * 4
            for d in dirs:
                for sub in range(2):
                    ps = gtile(f"g0d{d}s{sub}")
                    tiles[2 * d + sub] = ps
                    col = d * 784 + sub * GNT
                    out = ps[:, :GNT]
                    seqm = [(e_allT[:, 0, t64:t64 + N], w0e[:, 0, col:col + GNT]),
                            (e_allT[:69, 1, t64:t64 + N], w0e[:69, 1, col:col + GNT])]
                    for cb, (blk, cnt) in enumerate(((2 * d, 128), (2 * d + 1, 68))):
                        seqm.append((h0T[:cnt, blk, :], w0h[:cnt, cb, col:col + GNT]))
                    for i, (lh, rh) in enumerate(seqm):
                        mm(out=out, lhsT=lh, rhs=rh, start=(i == 0), stop=False)
            return tiles

        def l0_ctx_mms(tiles, parity):
            for d in range(2):
                for sub in range(2):
                    ps = tiles[2 * d + sub]
                    col = d * 784 + sub * GNT
                    out = ps[:, :GNT]
                    for k in range(4):
                        mm(out=out, lhsT=ctxT[:, parity, k, :],
                           rhs=w0c[:, k, col:col + GNT],
                           start=False, stop=(k == 3))

        def l1_mms(t):
            tiles = []
            for d in range(2):
                for sub in range(2):
                    ps = gtile(f"g1d{d}s{sub}")
                    tiles.append(ps)
                    col = d * 784 + sub * GNT
                    out = ps[:, :GNT]
                    seqm = []
                    for b, (r0, sz) in enumerate(HBLK):
                        szx = sz + 1 if b == 3 else sz  # include ones row
                        seqm.append((h0T[:szx, b, :], w1x[:szx, b, col:col + GNT]))
                    for cb, (blk, cnt) in enumerate(((2 * d, 128), (2 * d + 1, 68))):
                        seqm.append((h1T[:cnt, blk, :], w1h[:cnt, cb, col:col + GNT]))
                    last = len(seqm) - 1
                    for i, (lh, rh) in enumerate(seqm):
                        mm(out=out, lhsT=lh, rhs=rh, start=(i == 0), stop=(i == last))
            return tiles

        def gates_tanh(tiles, layer):
            Tg = work.tile([N, 4, GNT], F16, name=f"T{layer}", tag=f"T{layer}")
            for d in range(2):
                for sub in range(2):
                    nc.scalar.activation(out=Tg[:, 2 * d + sub, :],
                                         in_=tiles[2 * d + sub][:, :GNT],
                                         func=AF.Tanh)
            return Tg

        def cell_dve(Tg, layer):
            """fp16 cell math; returns hh [N, 2E] (h~ row layout)."""
            Ct = Ct0 if layer == 0 else Ct1
            T_i = Tg[:, 0::2, 0:E]
            T_f = Tg[:, 0::2, E:2 * E]
            T_o = Tg[:, 1::2, 0:E]
            T_g = Tg[:, 1::2, E:2 * E]
            u = work.tile([N, 2, E], F16, name="u", tag="u")
            sf = work.tile([N, 2, E], F16, name="sf", tag="sf")
            nc.vector.scalar_tensor_tensor(out=u, in0=T_i, scalar=1.0, in1=T_g,
                                           op0=OP.add, op1=OP.mult)
            nc.vector.tensor_scalar(out=sf, in0=T_f, scalar1=0.5, scalar2=0.5,
                                    op0=OP.mult, op1=OP.add)
            nc.vector.tensor_tensor(out=sf, in0=sf, in1=Ct, op=OP.mult)
            nc.vector.tensor_tensor(out=Ct, in0=u, in1=sf, op=OP.add)
            Tc = work.tile([N, 2, E], F16, name=f"Tc{layer}", tag="Tc")
            nc.scalar.activation(out=Tc, in_=Ct, func=AF.Tanh, scale=0.5)
            hh = work.tile([N, 2 * E], F16, name=f"h{layer}_", tag=f"h{layer}_")
            hhv = hh.rearrange("p (a b) -> p a b", a=2)
            nc.vector.scalar_tensor_tensor(out=hhv, in0=T_o, scalar=1.0, in1=Tc,
                                           op0=OP.add, op1=OP.mult)
            return hh

        def h_transpose(hh, hT, layer):
            tph = mvtile([128, 4, N], F16, f"tph{layer}")
            for b, (c0, w) in enumerate(HBLK):
                nc.tensor.transpose(out=tph[:w, b, :], in_=hh[:, c0:c0 + w],
                                    identity=idn16[0:N, 0:N])
                nc.vector.tensor_copy(out=hT[:w, b, :], in_=tph[:w, b, :])

        def lin_mms(parity):
            lps = mvtile([N, 512], F32, "lps")
            seqm = []
            for b, (r0, sz) in enumerate(HBLK):
                szx = sz + 1 if b == 3 else sz
                seqm.append((h1T[:szx, b, :], lin_sb[:szx, b, :]))
            for k in range(4):
                seqm.append((ctxT[:, parity, k, :], lin_sb[:, 4 + k, :]))
            for i, (lh, rh) in enumerate(seqm):
                mm(out=lps, lhsT=lh, rhs=rh, start=(i == 0),
                   stop=(i == len(seqm) - 1))
            return lps

        def lin_leaky(lps):
            a16 = work.tile([N, M], F16, name="a16", tag="a16")
            lk = work.tile([N, M], F16, name="lk", tag="lk")
            # leaky_relu(x) = max(x, 0.01x), exact; one PSUM input per op
            nc.vector.tensor_scalar(out=lk, in0=lps, scalar1=0.01,
                                    scalar2=None, op0=OP.mult)
            nc.vector.tensor_tensor(out=a16, in0=lps, in1=lk, op=OP.max)
            return a16

        def a_transpose(a16):
            tpa = mvtile([128, 4, N], F16, "tpa")
            for b in range(4):
                nc.tensor.transpose(out=tpa[:, b, :], in_=a16[:, 128 * b:128 * (b + 1)],
                                    identity=idn16[0:N, 0:N])
                nc.vector.tensor_copy(out=aT[:, b, :], in_=tpa[:, b, :])

        def vocab_A(t):
            """nt0 matmuls (5), exp+accum, raw-logit stash of cols 0..511."""
            vpsA = mvtile([N, 512], F32, "vpsA")
            v0, w = VOC_NT[0]
            for k in range(5):
                cnt = 128 if k < 4 else 1
                mm(out=vpsA, lhsT=aT[:cnt, k, :],
                   rhs=wp_sb[:cnt, k, v0:v0 + w], start=(k == 0), stop=(k == 4))
            xraw = work.tile([N, LRAW_W], F16, name="xraw", tag="xraw", bufs=2)
            xv = xraw.rearrange("p (a b) -> p a b", a=3)
            dump = work.tile([N, LRAW_W], F16, name="dump", tag="dump", bufs=2)
            dv = dump.rearrange("p (a b) -> p a b", a=3)
            s1 = tiny.tile([N, 1], F32, name="s1", tag="s1")
            nc.scalar.activation(out=dv[:, 0, :], in_=vpsA, func=AF.Exp,
                                 accum_out=s1)
            nc.vector.tensor_copy(out=xv[:, 0, :], in_=vpsA)
            return xraw, dump, s1

        def vocab_B(t, xraw, dump, s1):
            """nt1+nt2 matmuls (10), exp+accum, stash, s-total, DRAM spill."""
            vpsB = mvtile([N, 2, 512], F32, "vpsB")
            for sl, nt in enumerate((1, 2)):
                v0, w = VOC_NT[nt]
                for k in range(5):
                    cnt = 128 if k < 4 else 1
                    mm(out=vpsB[:, sl, :w], lhsT=aT[:cnt, k, :],
                       rhs=wp_sb[:cnt, k, v0:v0 + w], start=(k == 0), stop=(k == 4))
            xv = xraw.rearrange("p (a b) -> p a b", a=3)
            dv = dump.rearrange("p (a b) -> p a b", a=3)
            s2 = tiny.tile([N, 1], F32, name="s2", tag="s2")
            nc.scalar.activation(out=dv[:, 1, :], in_=vpsB[:, 0, :], func=AF.Exp,
                                 accum_out=s2)
            s3 = tiny.tile([N, 1], F32, name="s3", tag="s3")
            nc.scalar.activation(out=dv[:, 2, :476], in_=vpsB[:, 1, :476],
                                 func=AF.Exp, accum_out=s3)
            nc.vector.tensor_copy(out=xv[:, 1, :], in_=vpsB[:, 0, :])
            nc.vector.tensor_copy(out=xv[:, 2, :476], in_=vpsB[:, 1, :476])
            nc.vector.tensor_tensor(out=s2, in0=s2, in1=s3, op=OP.add)
            nc.vector.tensor_tensor(out=sAll[:, t:t + 1], in0=s1, in1=s2, op=OP.add)
            nc.sync.dma_start(out=d_lraw[t][:, :1500], in_=xraw[:, :1500])

        # ---------- initial context (writes parity 1) ----------
        craw, q0 = ctx_mvs(g_allT, 0, 1)
        g0_tiles = l0_eh_mms(0)          # fills the initial ctx-norm tail
        ctx_norm(craw, q0, 1)

        # ---------- software-pipelined steps ----------
        # Loop iteration t interleaves step t's critical path (gates0 -> cell0
        # -> gates1 -> cell1 -> ctx) with step t-1's vocab in the cell gaps
        # and step t+1's gates0-partial + step t's lin/aT/vocab-head in the
        # ctx-norm tail, so the PE stays fed through every serial chain.
        vA = None
        for t in range(n_steps):
            l0_ctx_mms(g0_tiles, (t - 1) % 2)            # finish gates0(t)
            Tg0 = gates_tanh(g0_tiles, 0)
            if t > 0:
                vocab_B(t - 1, *vA)                      # fills cell0 gap
            hh0 = cell_dve(Tg0, 0)
            h_transpose(hh0, h0T, 0)
            g1_tiles = l1_mms(t)
            Tg1 = gates_tanh(g1_tiles, 1)
            if t + 1 < n_steps:
                g0_tiles = l0_eh_mms(t + 1, dirs=(0,))   # fills cell1 gap
            hh1 = cell_dve(Tg1, 1)
            h_transpose(hh1, h1T, 1)
            craw, q = ctx_mvs(h1T, 2, 3)
            if t + 1 < n_steps:
                l0_eh_mms(t + 1, dirs=(1,), tiles=g0_tiles)  # tail fill
            lps = lin_mms((t - 1) % 2)                   # lin(t), tail fill
            ctx_norm(craw, q, t % 2)
            a16 = lin_leaky(lps)
            a_transpose(a16)                             # aT(t), tail fill
            vA = vocab_A(t)                              # vocab head, tail fill

        # trailing vocab for the last step
        vocab_B(n_steps - 1, *vA)

        mappool.release()

        # ---------- finale: AllReduce s, ln, subtract ----------
        nc.sync.dma_start(out=bass.AP(tensor=d_s_in.tensor, offset=0,
                                      ap=[[T, N], [1, T]]), in_=sAll)
        nc.gpsimd.collective_compute("AllReduce", OP.add, replica_groups=RG,
                                     ins=[d_s_in[:]], outs=[d_s_out[:]])
        finp = tc.alloc_tile_pool(name="finp", bufs=3)
        sg = state.tile([N, T], F32, name="sg")
        nc.sync.dma_start(out=sg, in_=bass.AP(tensor=d_s_out.tensor, offset=0,
                                              ap=[[T, N], [1, T]]))
        lns = state.tile([N, T], F32, name="lns")
        nc.scalar.activation(out=lns, in_=sg, func=AF.Ln)
        nlns = state.tile([N, T], F32, name="nlns")
        nc.vector.tensor_scalar(out=nlns, in0=lns, scalar1=-1.0, scalar2=None,
                                op0=OP.mult)
        for t in range(T):
            xst = finp.tile([N, LRAW_W], F16, name="xst", tag="xst")
            nc.sync.dma_start(out=xst[:, :1500], in_=d_lraw[t][:, :1500])
            ot = finp.tile([N, VS], F32, name="ot", tag="ot")
            if t % 2 == 0:
                nc.vector.tensor_scalar(out=ot, in0=xst[:, 0:VS],
                                        scalar1=lns[:, t:t + 1],
                                        scalar2=None, op0=OP.subtract)
            else:
                nc.scalar.activation(out=ot, in_=xst[:, 0:VS], func=AF.Identity,
                                     bias=nlns[:, t:t + 1])
            nc.sync.dma_start(out=d_out[t], in_=ot)
        finp.release()
        for p in (psum, tiny, work, state, wpool):
            p.release()
    return nc


_CACHED = {}


def _build_nc(n_steps=T):
    key = ("nc", n_steps)
    if key not in _CACHED:
        nc = bacc.Bacc("TRN2", target_bir_lowering=False, debug=False,
                       num_devices=NCORES)
        build(nc, n_steps)
        nc.compile()
        _CACHED[key] = nc
    return _CACHED[key]


def run(inputs, trace=False):
    nc = _build_nc()
    in_maps = prepare_inputs(inputs)
    res = run_bass_kernel_spmd(nc, in_maps, list(range(NCORES)), trace=trace)
    out = np.concatenate([res.results[r]["out_logits"] for r in range(NCORES)],
                         axis=2)
    return out.astype(np.float32), res


def kernel(**inputs):
    out, _ = run(inputs, trace=False)
    return out
